# revision 21
# baseline (speedup 1.0000x reference)
"""MLA attention (DeepSeek-style) Trainium2 Bass kernel, 8-core SPMD.

Sharding: core c handles batch b = c//4 and head-group g = c%4 (4 of 16 heads).
All low-rank projections are fused on the host (Wq_down@Wq_up etc.), so every
core runs a single head-parallel projection x @ Wqk [D, 1024] (per-head
[q_nope|q_pe] / [k_nope|k_pe] column tiles) + x @ Wv [D, 512] with ZERO
replicated work, then causal flash attention for its 4 heads and a partial
o-projection. Host sums the 4 partial o-projections per batch.

Device dataflow (per core, transposed-activation layout, S processed in 4
chunks of 512):
  xT (host-tiled, bf16) -> per-head qT/kT [128=HD, S] bf16 tiles straight from
  PSUM (nope rows 0:64, rope rows 64:128), RoPE via host-baked cos/sin tables
  -> V in natural layout via swapped-operand matmuls -> causal flash attention
  per head: scoresT [j,i] matmuls with diagonal tiles shrunk to the unmasked
  query range, exp on ScalarE (scale fused), unnormalized attnout + ones-matmul
  row sums, normalize by broadcast reciprocal -> o-projection -> partial
  [S, D] bf16 out (pre-tiled layout).

Every DMA batch gets its own SBUF tile (dependency tracking is per-tile, so
shared tiles would serialize consumers on the LAST dma). Weights stream on the
gpsimd ring, x on the sync ring, tables/outputs on the scalar ring.
"""

import numpy as np
import ml_dtypes

import concourse.bacc as bacc
import concourse.mybir as mybir
import concourse.tile as tile
from concourse.bass_utils import run_bass_kernel_spmd

F32 = mybir.dt.float32
BF16 = mybir.dt.bfloat16

B, S, D = 2, 2048, 2048
H, HD = 16, 128
RD, ND = 64, 64
KVR, QR = 512, 1024
BASE = 10000.0
HLOC = 4                 # heads per core
CHUNK = 512
NCHUNK = S // CHUNK      # 4
P = 128
DT = D // P              # 16 contraction tiles
NCT = 2 * HLOC           # 8 projection c-tiles (4 q heads + 4 k heads)
SCALE = HD ** -0.5

_BF16 = ml_dtypes.bfloat16


def _build():
    nc = bacc.Bacc("TRN2", target_bir_lowering=False, debug=False)

    xt = nc.dram_tensor("xt", [P, NCHUNK * DT * CHUNK], BF16,
                        kind="ExternalInput").ap()
    wqk = nc.dram_tensor("wqk", [P, NCT * DT * P], BF16,
                         kind="ExternalInput").ap()
    wv = nc.dram_tensor("wv", [P, DT * HLOC * HD], BF16,
                        kind="ExternalInput").ap()
    wo = nc.dram_tensor("wo", [P, HLOC * D], BF16, kind="ExternalInput").ap()
    cosr = nc.dram_tensor("cosr", [RD, S], F32, kind="ExternalInput").ap()
    sinr = nc.dram_tensor("sinr", [RD, S], F32, kind="ExternalInput").ap()
    maskd = nc.dram_tensor("maskd", [P, P], BF16, kind="ExternalInput").ap()
    # output pre-tiled [p, st, d]: fat 4KB-per-partition DMA descriptors
    o_part = nc.dram_tensor("o_part", [P, (S // P) * D], BF16,
                            kind="ExternalOutput").ap()

    xt_r = xt.rearrange("p (ic hf dt s) -> p ic hf dt s",
                        ic=NCHUNK, hf=2, dt=DT // 2)
    wqk_r = wqk.rearrange("p (ct dt c) -> p ct dt c", ct=NCT, dt=DT)
    wv_r = wv.rearrange("p (hf dt c) -> p hf dt c", hf=2, dt=DT // 2)
    wo_r = wo.rearrange("p (kt d) -> p kt d", kt=HLOC)
    o_r = o_part.rearrange("p (st d) -> p st d", st=S // P)  # [128, 16, 2048]

    with tile.TileContext(nc) as tc:
        with (
            tc.tile_pool(name="persist", bufs=1) as pp,
            tc.tile_pool(name="acts", bufs=2) as ap_,
            tc.tile_pool(name="rope", bufs=2) as rp,
            tc.tile_pool(name="attn", bufs=2) as atp,
            tc.tile_pool(name="outp", bufs=2) as op_,
            tc.tile_pool(name="aoutp", bufs=2) as aop,
            tc.tile_pool(name="psA", bufs=2, space="PSUM") as psA,
            tc.tile_pool(name="psS", bufs=2, space="PSUM") as psS,
            tc.tile_pool(name="psD", bufs=2, space="PSUM") as psD,
            tc.tile_pool(name="psO", bufs=2, space="PSUM") as psO,
        ):
            # ---------------- persistent tiles (one per DMA batch) ----------
            kTs = [pp.tile([P, S], BF16, tag=f"kT{h}", name=f"kT{h}")
                   for h in range(HLOC)]
            vnat = pp.tile([P, S // P, HLOC * HD], BF16, tag="vnat")
            masks = pp.tile([P, P], BF16, tag="masks")
            ones = pp.tile([P, P], BF16, tag="ones")
            wqkts = [pp.tile([P, DT, P], BF16, tag=f"wqk{ct}", name=f"wqk{ct}")
                     for ct in range(NCT)]
            wvts = [pp.tile([P, DT // 2, HLOC * HD], BF16, tag=f"wv{i}",
                            name=f"wv{i}") for i in range(2)]
            wots = [pp.tile([P, D], BF16, tag=f"wo{kt}", name=f"wo{kt}")
                    for kt in range(HLOC)]
            cos_t = pp.tile([P, S], F32, tag="cos")
            sin_t = pp.tile([P, S], F32, tag="sin")
            dummy = pp.tile([P, 4], BF16, tag="dummy")

            nc.vector.memset(ones[:], 1.0)

            def o_proj(ic, aouts, pre_triggers=()):
                for trg in pre_triggers:
                    trg()
                for st in range(CHUNK // P):
                    osb = op_.tile([P, D // CHUNK, CHUNK], BF16, tag="osb")
                    for dc in range(D // CHUNK):
                        ps = psA.tile([P, CHUNK], F32, tag="psA")
                        for kt in range(HLOC):
                            nc.tensor.matmul(
                                ps[:], aouts[kt][:, P * st:P * (st + 1)],
                                wots[kt][:, CHUNK * dc:CHUNK * (dc + 1)],
                                start=(kt == 0), stop=(kt == HLOC - 1))
                        if dc % 2 == 0:
                            nc.vector.tensor_copy(osb[:, dc, :], ps[:])
                        else:
                            nc.scalar.copy(osb[:, dc, :], ps[:])
                    nc.scalar.dma_start(
                        o_r[:, ic * (CHUNK // P) + st, :], osb[:])

            def rope_store(ps_pe, dst_pe, cos_c, sin_c):
                """ps_pe: [64, CHUNK] psum AP at partition base 64 (pre-rope pe
                rows of one head). 4 DVE ops; sign-baked sin tables make the
                NeoX rotation a mult/mult/mult/add. dst_pe = rows [64:128]."""
                b = 64
                tmp = rp.tile([P, CHUNK], F32, tag="ropetmp")
                scr = rp.tile([P, CHUNK], F32, tag="ropescr")
                nc.vector.tensor_tensor(tmp[b:b + 32, :], ps_pe[32:64, :],
                                        sin_c[b:b + 32, :], mybir.AluOpType.mult)
                nc.vector.tensor_tensor(tmp[b + 32:b + 64, :], ps_pe[0:32, :],
                                        sin_c[b + 32:b + 64, :],
                                        mybir.AluOpType.mult)
                nc.vector.tensor_tensor(scr[b:b + 64, :], ps_pe[:],
                                        cos_c[b:b + 64, :], mybir.AluOpType.mult)
                nc.vector.tensor_tensor(dst_pe, scr[b:b + 64, :],
                                        tmp[b:b + 64, :], mybir.AluOpType.add)

            # ---------------- chunk loop ----------------
            # xc is prefetched one chunk ahead: chunk 0 on the otherwise-idle
            # sync ring, later chunks triggered from scalar-engine positions
            # that execute well before the data is needed
            cur_xcs = [ap_.tile([P, DT // 2, CHUNK], BF16, tag=f"xc{i}",
                                name=f"xc{i}") for i in range(2)]
            nc.sync.dma_start(cur_xcs[0][:], xt_r[:, 0, 0])
            nc.sync.dma_start(cur_xcs[1][:], xt_r[:, 0, 1])

            for ic in range(NCHUNK):
                sl = slice(ic * CHUNK, (ic + 1) * CHUNK)

                xcs = cur_xcs
                if ic + 1 < NCHUNK:
                    next_xcs = [ap_.tile([P, DT // 2, CHUNK], BF16,
                                         tag=f"xc{i}", name=f"xc{i}")
                                for i in range(2)]
                    cur_xcs = next_xcs

                def xsl(dt_, cols=slice(None), xcs=xcs):
                    return xcs[dt_ // (DT // 2)][:, dt_ % (DT // 2), cols]

                if ic == 0:
                    # critical set only — everything else is trigger-delayed
                    # via vector-engine emission placement below, so it can't
                    # steal SDMA packet slots from these
                    nc.gpsimd.dma_start(wqkts[0][:], wqk_r[:, 0])
                    nc.gpsimd.dma_start(wqkts[1][:], wqk_r[:, 1])
                    nc.scalar.dma_start(cos_t[64:128, :], cosr[:])
                    nc.scalar.dma_start(sin_t[64:128, :], sinr[:])
                    nc.scalar.dma_start(masks[:], maskd[:])

                cos_c = cos_t[:, sl]
                sin_c = sin_t[:, sl]

                # ---- projection: c-tile ct = head [nope64 | pe64] ----
                # ct 0..3 -> q heads, ct 4..7 -> k heads (identical rope)
                qTis = [ap_.tile([P, CHUNK], BF16, tag=f"qTi{h}", name=f"qTi{h}")
                        for h in range(HLOC)]
                for ct in range(NCT):
                    ps = psA.tile([P, CHUNK], F32, tag="psA")
                    for dt_ in range(DT):
                        nc.tensor.matmul(
                            ps[:], wqkts[ct][:, dt_, :], xsl(dt_),
                            start=(dt_ == 0), stop=(dt_ == DT - 1))
                    if ct < HLOC:
                        dst_nope = qTis[ct][0:64, :]
                        dst_pe = qTis[ct][64:128, :]
                    else:
                        dst_nope = kTs[ct - HLOC][0:64, sl]
                        dst_pe = kTs[ct - HLOC][64:128, sl]
                    nc.vector.tensor_copy(dst_nope, ps[0:64, :])
                    rope_store(ps[64:128, :], dst_pe, cos_c, sin_c)
                    if ic == 0 and ct == 0:
                        # gate the remaining loads behind rope(ct0): the
                        # dummy copy makes the scalar engine (in-order) hold
                        # these triggers until the pipeline is rolling, so
                        # they can't steal SDMA packet slots from the
                        # chunk-0 critical loads
                        nc.scalar.copy(dummy[0:1, 0:1], qTis[0][64:65, 0:1])
                        for c2 in range(2, NCT):
                            nc.scalar.dma_start(wqkts[c2][:], wqk_r[:, c2])
                        nc.scalar.dma_start(wvts[0][:], wv_r[:, 0])
                        nc.scalar.dma_start(wvts[1][:], wv_r[:, 1])

                if ic == 0:
                    # Wo + next x chunk, gated behind rope(ct7)
                    nc.scalar.copy(dummy[0:1, 1:2], kTs[3][64:65, 0:1])
                    for kt in range(HLOC):
                        nc.scalar.dma_start(wots[kt][:], wo_r[:, kt])
                    nc.scalar.dma_start(next_xcs[0][:], xt_r[:, 1, 0])
                    nc.scalar.dma_start(next_xcs[1][:], xt_r[:, 1, 1])

                # ---- v natural [CHUNK, 512]: x seq-tile stationary ----
                for st in range(CHUNK // P):
                    ps = psA.tile([P, HLOC * HD], F32, tag="psA")
                    for dt_ in range(DT):
                        nc.tensor.matmul(
                            ps[:], xsl(dt_, slice(P * st, P * (st + 1))),
                            wvts[dt_ // (DT // 2)][:, dt_ % (DT // 2), :],
                            start=(dt_ == 0), stop=(dt_ == DT - 1))
                    nc.vector.tensor_copy(vnat[:, ic * (CHUNK // P) + st, :],
                                          ps[:])

                # ---- o-projection of the PREVIOUS chunk: PE work to cover
                # the DVE rope/normalize backlog of this chunk's projections.
                # Its scalar-stream position also paces the xc prefetch for
                # chunk ic+1.
                if ic > 0:
                    trgs = ()
                    if ic + 1 < NCHUNK:
                        trgs = (
                            lambda nx=next_xcs: (
                                nc.scalar.dma_start(nx[0][:],
                                                    xt_r[:, ic + 1, 0]),
                                nc.scalar.dma_start(nx[1][:],
                                                    xt_r[:, ic + 1, 1]),
                            ),
                        )
                    o_proj(ic - 1, prev_aouts, trgs)

                # ---- attention for this query chunk ----
                # diagonal j-tiles shrink to queries >= P*r (the rest are
                # fully masked and contribute exact zeros by omission)
                aouts = [aop.tile([P, CHUNK], BF16, tag=f"aout{h}", name=f"aout{h}")
                         for h in range(HLOC)]
                jt_max = (ic + 1) * (CHUNK // P)
                for h in range(HLOC):
                    psd = psD.tile([P, CHUNK], F32, tag="psD")
                    pso = psO.tile([P, CHUNK], F32, tag="psO")
                    for jt in range(jt_max):
                        r = jt - ic * (CHUNK // P)
                        q0 = P * r if r > 0 else 0
                        pss = psS.tile([P, CHUNK], F32, tag="psS")
                        nc.tensor.matmul(
                            pss[:, q0:], kTs[h][:, P * jt:P * (jt + 1)],
                            qTis[h][:, q0:], start=True, stop=True)
                        at = atp.tile([P, CHUNK], BF16, tag="attnT")
                        nc.scalar.activation(
                            at[:, q0:], pss[:, q0:],
                            mybir.ActivationFunctionType.Exp, scale=SCALE)
                        if r >= 0:  # triangular mask on the diagonal subtile
                            nc.vector.tensor_tensor(
                                at[:, q0:q0 + P], at[:, q0:q0 + P], masks[:],
                                mybir.AluOpType.mult)
                        nc.tensor.matmul(psd[:, q0:], ones[:], at[:, q0:],
                                         start=(jt == 0), stop=(jt == jt_max - 1))
                        nc.tensor.matmul(
                            pso[:, q0:], vnat[:, jt, HD * h:HD * (h + 1)],
                            at[:, q0:],
                            start=(jt == 0), stop=(jt == jt_max - 1))
                    rec = atp.tile([P, CHUNK], F32, tag="recip")
                    nc.vector.reciprocal_approx_fast(rec[:], psd[:])
                    nc.vector.tensor_tensor(aouts[h][:], pso[:], rec[:],
                                            mybir.AluOpType.mult)
                prev_aouts = aouts

            o_proj(NCHUNK - 1, prev_aouts)
    nc.compile()
    return nc


_NC = None


def _get_nc():
    global _NC
    if _NC is None:
        _NC = _build()
    return _NC


def _host_prep(x, Wq_down, Wq_up, Wq_rope, Wkv_down, Wk_up, Wk_rope, Wv_up, Wo):
    """Build the 8 per-core input maps (all host-side layout prep)."""
    # rope tables for SBUF partition rows 64:128 (the pe rows), NeoX sign
    # baked into sin
    half = RD // 2
    inv_freq = 1.0 / (BASE ** (np.arange(half, dtype=np.float64) / half))
    ang = np.arange(S, dtype=np.float64)[None, :] * inv_freq[:, None]  # [32, S]
    cos32 = np.cos(ang)
    sin32 = np.sin(ang)
    cosr = np.tile(cos32, (2, 1)).astype(np.float32)                   # [64,S]
    sinr = np.concatenate([-sin32, sin32], 0).astype(np.float32)

    # triangular mask for the 128x128 diagonal subtile: key p <= query i
    pidx = np.arange(P)[:, None]
    iidx = np.arange(P)[None, :]
    maskd = (pidx <= iidx).astype(_BF16)

    # fuse the low-rank compositions once, in f32
    Wfq = Wq_down @ Wq_up        # [D, H*ND]
    Wfqr = Wq_down @ Wq_rope     # [D, H*RD]
    Wfk = Wkv_down @ Wk_up       # [D, H*ND]
    Wfv = Wkv_down @ Wv_up       # [D, H*HD]

    # per-batch pre-tiled x^T: [p, ic, dt, s]
    xts = [np.ascontiguousarray(
        x[b].reshape(NCHUNK, CHUNK, DT, P).transpose(3, 0, 2, 1)
    ).reshape(P, -1).astype(_BF16) for b in range(B)]

    in_maps = []
    for c in range(8):
        b, g = divmod(c, 4)
        heads = range(HLOC * g, HLOC * (g + 1))
        Wqk = np.empty((D, NCT * P), np.float32)
        for i, h in enumerate(heads):
            q0, k0 = i * HD, HLOC * HD + i * HD
            Wqk[:, q0:q0 + ND] = Wfq[:, h * ND:(h + 1) * ND]
            Wqk[:, q0 + ND:q0 + HD] = Wfqr[:, h * RD:(h + 1) * RD]
            Wqk[:, k0:k0 + ND] = Wfk[:, h * ND:(h + 1) * ND]
            Wqk[:, k0 + ND:k0 + HD] = Wk_rope[:, h * RD:(h + 1) * RD]
        Wv = Wfv[:, g * HLOC * HD:(g + 1) * HLOC * HD]
        Wop = Wo[g * HLOC * HD:(g + 1) * HLOC * HD, :]
        in_maps.append({
            "xt": xts[b],
            "wqk": np.ascontiguousarray(
                Wqk.reshape(DT, P, NCT, P).transpose(1, 2, 0, 3)
            ).reshape(P, -1).astype(_BF16),
            "wv": np.ascontiguousarray(
                Wv.reshape(DT, P, HLOC * HD).transpose(1, 0, 2)
            ).reshape(P, -1).astype(_BF16),
            "wo": np.ascontiguousarray(
                Wop.reshape(HLOC, P, D).transpose(1, 0, 2)
            ).reshape(P, -1).astype(_BF16),
            "cosr": cosr,
            "sinr": sinr,
            "maskd": maskd,
        })
    return in_maps


def kernel(x, Wq_down, Wq_up, Wq_rope, Wkv_down, Wk_up, Wk_rope, Wv_up, Wo,
           _trace=False, _trace_kwargs=None):
    x = np.asarray(x, dtype=np.float32)
    args = [np.asarray(a, dtype=np.float32) for a in
            (Wq_down, Wq_up, Wq_rope, Wkv_down, Wk_up, Wk_rope, Wv_up, Wo)]
    in_maps = _host_prep(x, *args)
    nc = _get_nc()
    res = run_bass_kernel_spmd(nc, in_maps, core_ids=list(range(8)),
                               trace=_trace, **(_trace_kwargs or {}))
    kernel._last_results = res
    out = np.zeros((B, S, D), np.float32)
    for c in range(8):
        # un-tile [p, st, d] -> [st*128+p, d]
        part = res.results[c]["o_part"].reshape(P, S // P, D)
        out[c // 4] += part.transpose(1, 0, 2).reshape(S, D).astype(np.float32)
    return out


# revision 22
# speedup vs baseline: 1.1880x; 1.1880x over previous
"""MLA attention (DeepSeek-style) Trainium2 Bass kernel, 8-core SPMD.

Sharding: core c handles batch b = c//4 and head-group g = c%4 (4 of 16 heads).
All low-rank projections are fused on the host (Wq_down@Wq_up etc.), so every
core runs a single head-parallel projection x @ Wqk [D, 1024] (per-head
[q_nope|q_pe] / [k_nope|k_pe] column tiles) + x @ Wv [D, 512] with ZERO
replicated work, then causal flash attention for its 4 heads and a partial
o-projection. Host sums the 4 partial o-projections per batch.

Device dataflow (per core, transposed-activation layout, S processed in 4
chunks of 512):
  xT (host-tiled, bf16) -> per-head qT/kT [128=HD, S] bf16 tiles straight from
  PSUM (nope rows 0:64, rope rows 64:128), RoPE via host-baked cos/sin tables
  -> V in natural layout via swapped-operand matmuls -> causal flash attention
  per head: scoresT [j,i] matmuls with diagonal tiles shrunk to the unmasked
  query range, exp on ScalarE (scale fused) issued one (head,jt) pair ahead of
  its ones/av consumers (software pipelining - no head-boundary PE bubbles),
  unnormalized attnout + ones-matmul row sums, normalize by broadcast
  reciprocal -> o-projection -> partial [S, D] bf16 out (pre-tiled layout).

Every DMA batch gets its own SBUF tile (dependency tracking is per-tile, so
shared tiles would serialize consumers on the LAST dma). ALL loads are issued
up front: mid-compute DMA measurably slows matmuls via SBUF port contention,
so paying ~20us of saturated-DMA startup buys a clean compute window.
"""

import numpy as np
import ml_dtypes

import concourse.bacc as bacc
import concourse.mybir as mybir
import concourse.tile as tile
from concourse.bass_utils import run_bass_kernel_spmd

F32 = mybir.dt.float32
BF16 = mybir.dt.bfloat16

B, S, D = 2, 2048, 2048
H, HD = 16, 128
RD, ND = 64, 64
KVR, QR = 512, 1024
BASE = 10000.0
HLOC = 4                 # heads per core
CHUNK = 512
NCHUNK = S // CHUNK      # 4
P = 128
DT = D // P              # 16 contraction tiles
NCT = 2 * HLOC           # 8 projection c-tiles (4 q heads + 4 k heads)
SCALE = HD ** -0.5

_BF16 = ml_dtypes.bfloat16


def _build():
    nc = bacc.Bacc("TRN2", target_bir_lowering=False, debug=False)

    xt = nc.dram_tensor("xt", [P, NCHUNK * DT * CHUNK], BF16,
                        kind="ExternalInput").ap()
    wqk = nc.dram_tensor("wqk", [P, NCT * DT * P], BF16,
                         kind="ExternalInput").ap()
    wv = nc.dram_tensor("wv", [P, DT * HLOC * HD], BF16,
                        kind="ExternalInput").ap()
    wo = nc.dram_tensor("wo", [P, HLOC * D], BF16, kind="ExternalInput").ap()
    cosr = nc.dram_tensor("cosr", [RD, S], F32, kind="ExternalInput").ap()
    sinr = nc.dram_tensor("sinr", [RD, S], F32, kind="ExternalInput").ap()
    maskd = nc.dram_tensor("maskd", [P, P], BF16, kind="ExternalInput").ap()
    # output pre-tiled [p, st, d]: fat 4KB-per-partition DMA descriptors
    o_part = nc.dram_tensor("o_part", [P, (S // P) * D], BF16,
                            kind="ExternalOutput").ap()

    xt_r = xt.rearrange("p (ic hf dt s) -> p ic hf dt s",
                        ic=NCHUNK, hf=2, dt=DT // 2)
    wqk_r = wqk.rearrange("p (ct dt c) -> p ct dt c", ct=NCT, dt=DT)
    wv_r = wv.rearrange("p (hf dt c) -> p hf dt c", hf=2, dt=DT // 2)
    wo_r = wo.rearrange("p (kt d) -> p kt d", kt=HLOC)
    o_r = o_part.rearrange("p (st d) -> p st d", st=S // P)  # [128, 16, 2048]

    with tile.TileContext(nc) as tc:
        with (
            tc.tile_pool(name="persist", bufs=1) as pp,
            tc.tile_pool(name="acts", bufs=2) as ap_,
            tc.tile_pool(name="rope", bufs=2) as rp,
            tc.tile_pool(name="attn", bufs=2) as atp,
            tc.tile_pool(name="outp", bufs=2) as op_,
            tc.tile_pool(name="aoutp", bufs=2) as aop,
            tc.tile_pool(name="psA", bufs=2, space="PSUM") as psA,
            tc.tile_pool(name="psS", bufs=2, space="PSUM") as psS,
            tc.tile_pool(name="psD", bufs=2, space="PSUM") as psD,
            tc.tile_pool(name="psO", bufs=2, space="PSUM") as psO,
        ):
            # ---------------- persistent tiles (one per DMA batch) ----------
            kTs = [pp.tile([P, S], BF16, tag=f"kT{h}", name=f"kT{h}")
                   for h in range(HLOC)]
            vnat = pp.tile([P, S // P, HLOC * HD], BF16, tag="vnat")
            masks = pp.tile([P, P], BF16, tag="masks")
            ones = pp.tile([P, P], BF16, tag="ones")
            wqkts = [pp.tile([P, DT, P], BF16, tag=f"wqk{ct}", name=f"wqk{ct}")
                     for ct in range(NCT)]
            wvts = [pp.tile([P, DT // 2, HLOC * HD], BF16, tag=f"wv{i}",
                            name=f"wv{i}") for i in range(2)]
            wots = [pp.tile([P, D], BF16, tag=f"wo{kt}", name=f"wo{kt}")
                    for kt in range(HLOC)]
            cos_t = pp.tile([P, S], F32, tag="cos")
            sin_t = pp.tile([P, S], F32, tag="sin")

            nc.vector.memset(ones[:], 1.0)

            def o_proj(ic, aouts):
                for st in range(CHUNK // P):
                    osb = op_.tile([P, D // CHUNK, CHUNK], BF16, tag="osb")
                    for dc in range(D // CHUNK):
                        ps = psA.tile([P, CHUNK], F32, tag="psA")
                        for kt in range(HLOC):
                            nc.tensor.matmul(
                                ps[:], aouts[kt][:, P * st:P * (st + 1)],
                                wots[kt][:, CHUNK * dc:CHUNK * (dc + 1)],
                                start=(kt == 0), stop=(kt == HLOC - 1))
                        if dc % 2 == 0:
                            nc.vector.tensor_copy(osb[:, dc, :], ps[:])
                        else:
                            nc.scalar.copy(osb[:, dc, :], ps[:])
                    nc.scalar.dma_start(
                        o_r[:, ic * (CHUNK // P) + st, :], osb[:])

            def rope_store(ps_pe, dst_pe, cos_c, sin_c):
                """ps_pe: [64, CHUNK] psum AP at partition base 64 (pre-rope pe
                rows of one head). 4 DVE ops; sign-baked sin tables make the
                NeoX rotation a mult/mult/mult/add. dst_pe = rows [64:128]."""
                b = 64
                tmp = rp.tile([P, CHUNK], F32, tag="ropetmp")
                scr = rp.tile([P, CHUNK], F32, tag="ropescr")
                nc.vector.tensor_tensor(tmp[b:b + 32, :], ps_pe[32:64, :],
                                        sin_c[b:b + 32, :], mybir.AluOpType.mult)
                nc.vector.tensor_tensor(tmp[b + 32:b + 64, :], ps_pe[0:32, :],
                                        sin_c[b + 32:b + 64, :],
                                        mybir.AluOpType.mult)
                nc.vector.tensor_tensor(scr[b:b + 64, :], ps_pe[:],
                                        cos_c[b:b + 64, :], mybir.AluOpType.mult)
                nc.vector.tensor_tensor(dst_pe, scr[b:b + 64, :],
                                        tmp[b:b + 64, :], mybir.AluOpType.add)

            # ---------------- chunk loop ----------------
            for ic in range(NCHUNK):
                sl = slice(ic * CHUNK, (ic + 1) * CHUNK)

                xcs = [ap_.tile([P, DT // 2, CHUNK], BF16, tag=f"xc{i}",
                                name=f"xc{i}") for i in range(2)]
                nc.sync.dma_start(xcs[0][:], xt_r[:, ic, 0])
                nc.sync.dma_start(xcs[1][:], xt_r[:, ic, 1])

                def xsl(dt_, cols=slice(None), xcs=xcs):
                    return xcs[dt_ // (DT // 2)][:, dt_ % (DT // 2), cols]

                if ic == 0:
                    for ct in range(NCT):
                        nc.gpsimd.dma_start(wqkts[ct][:], wqk_r[:, ct])
                    nc.scalar.dma_start(cos_t[64:128, :], cosr[:])
                    nc.scalar.dma_start(sin_t[64:128, :], sinr[:])
                    nc.scalar.dma_start(masks[:], maskd[:])
                    nc.gpsimd.dma_start(wvts[0][:], wv_r[:, 0])
                    nc.gpsimd.dma_start(wvts[1][:], wv_r[:, 1])

                cos_c = cos_t[:, sl]
                sin_c = sin_t[:, sl]

                # ---- projection: c-tile ct = head [nope64 | pe64] ----
                # ct 0..3 -> q heads, ct 4..7 -> k heads (identical rope)
                qTis = [ap_.tile([P, CHUNK], BF16, tag=f"qTi{h}",
                                 name=f"qTi{h}") for h in range(HLOC)]
                for ct in range(NCT):
                    ps = psA.tile([P, CHUNK], F32, tag="psA")
                    for dt_ in range(DT):
                        nc.tensor.matmul(
                            ps[:], wqkts[ct][:, dt_, :], xsl(dt_),
                            start=(dt_ == 0), stop=(dt_ == DT - 1))
                    if ct < HLOC:
                        dst_nope = qTis[ct][0:64, :]
                        dst_pe = qTis[ct][64:128, :]
                    else:
                        dst_nope = kTs[ct - HLOC][0:64, sl]
                        dst_pe = kTs[ct - HLOC][64:128, sl]
                    nc.vector.tensor_copy(dst_nope, ps[0:64, :])
                    rope_store(ps[64:128, :], dst_pe, cos_c, sin_c)

                if ic == 0:
                    # resident Wo load, deferred so it doesn't crowd the
                    # critical first-chunk x/weight DMAs
                    for kt in range(HLOC):
                        nc.gpsimd.dma_start(wots[kt][:], wo_r[:, kt])

                # ---- v natural [CHUNK, 512]: x seq-tile stationary ----
                for st in range(CHUNK // P):
                    ps = psA.tile([P, HLOC * HD], F32, tag="psA")
                    for dt_ in range(DT):
                        nc.tensor.matmul(
                            ps[:], xsl(dt_, slice(P * st, P * (st + 1))),
                            wvts[dt_ // (DT // 2)][:, dt_ % (DT // 2), :],
                            start=(dt_ == 0), stop=(dt_ == DT - 1))
                    nc.vector.tensor_copy(vnat[:, ic * (CHUNK // P) + st, :],
                                          ps[:])

                # ---- o-projection of the PREVIOUS chunk: PE work to cover
                # the DVE rope/normalize backlog of this chunk's projections
                if ic > 0:
                    o_proj(ic - 1, prev_aouts)

                # ---- attention for this query chunk ----
                # diagonal j-tiles shrink to queries >= P*r (the rest are
                # fully masked and contribute exact zeros by omission);
                # score+exp issue one (h, jt) pair ahead of ones/av
                aouts = [aop.tile([P, CHUNK], BF16, tag=f"aout{h}",
                                  name=f"aout{h}") for h in range(HLOC)]
                jt_max = (ic + 1) * (CHUNK // P)

                def issue_score(h, jt):
                    r = jt - ic * (CHUNK // P)
                    q0 = P * r if r > 0 else 0
                    pss = psS.tile([P, CHUNK], F32, tag="psS")
                    nc.tensor.matmul(
                        pss[:, q0:], kTs[h][:, P * jt:P * (jt + 1)],
                        qTis[h][:, q0:], start=True, stop=True)
                    at = atp.tile([P, CHUNK], BF16, tag="attnT")
                    nc.scalar.activation(
                        at[:, q0:], pss[:, q0:],
                        mybir.ActivationFunctionType.Exp, scale=SCALE)
                    if r >= 0:  # triangular mask on the diagonal subtile
                        nc.vector.tensor_tensor(
                            at[:, q0:q0 + P], at[:, q0:q0 + P], masks[:],
                            mybir.AluOpType.mult)
                    return at, q0

                pairs = [(h, jt) for h in range(HLOC) for jt in range(jt_max)]
                pending = {pairs[0]: issue_score(*pairs[0])}
                psd = pso = None
                for idx, (h, jt) in enumerate(pairs):
                    if idx + 1 < len(pairs):
                        pending[pairs[idx + 1]] = issue_score(*pairs[idx + 1])
                    at, q0 = pending.pop((h, jt))
                    if jt == 0:
                        psd = psD.tile([P, CHUNK], F32, tag="psD")
                        pso = psO.tile([P, CHUNK], F32, tag="psO")
                    nc.tensor.matmul(psd[:, q0:], ones[:], at[:, q0:],
                                     start=(jt == 0), stop=(jt == jt_max - 1))
                    nc.tensor.matmul(
                        pso[:, q0:], vnat[:, jt, HD * h:HD * (h + 1)],
                        at[:, q0:],
                        start=(jt == 0), stop=(jt == jt_max - 1))
                    if jt == jt_max - 1:
                        rec = atp.tile([P, CHUNK], F32, tag="recip")
                        nc.vector.reciprocal_approx_fast(rec[:], psd[:])
                        nc.vector.tensor_tensor(aouts[h][:], pso[:], rec[:],
                                                mybir.AluOpType.mult)
                prev_aouts = aouts

            o_proj(NCHUNK - 1, prev_aouts)
    nc.compile()
    return nc


_NC = None


def _get_nc():
    global _NC
    if _NC is None:
        _NC = _build()
    return _NC


def _host_prep(x, Wq_down, Wq_up, Wq_rope, Wkv_down, Wk_up, Wk_rope, Wv_up, Wo):
    """Build the 8 per-core input maps (all host-side layout prep)."""
    # rope tables for SBUF partition rows 64:128 (the pe rows), NeoX sign
    # baked into sin
    half = RD // 2
    inv_freq = 1.0 / (BASE ** (np.arange(half, dtype=np.float64) / half))
    ang = np.arange(S, dtype=np.float64)[None, :] * inv_freq[:, None]  # [32, S]
    cos32 = np.cos(ang)
    sin32 = np.sin(ang)
    cosr = np.tile(cos32, (2, 1)).astype(np.float32)                   # [64,S]
    sinr = np.concatenate([-sin32, sin32], 0).astype(np.float32)

    # triangular mask for the 128x128 diagonal subtile: key p <= query i
    pidx = np.arange(P)[:, None]
    iidx = np.arange(P)[None, :]
    maskd = (pidx <= iidx).astype(_BF16)

    # fuse the low-rank compositions once, in f32
    Wfq = Wq_down @ Wq_up        # [D, H*ND]
    Wfqr = Wq_down @ Wq_rope     # [D, H*RD]
    Wfk = Wkv_down @ Wk_up       # [D, H*ND]
    Wfv = Wkv_down @ Wv_up       # [D, H*HD]

    # per-batch pre-tiled x^T: [p, ic, dt, s]
    xts = [np.ascontiguousarray(
        x[b].reshape(NCHUNK, CHUNK, DT, P).transpose(3, 0, 2, 1)
    ).reshape(P, -1).astype(_BF16) for b in range(B)]

    in_maps = []
    for c in range(8):
        b, g = divmod(c, 4)
        heads = range(HLOC * g, HLOC * (g + 1))
        Wqk = np.empty((D, NCT * P), np.float32)
        for i, h in enumerate(heads):
            q0, k0 = i * HD, HLOC * HD + i * HD
            Wqk[:, q0:q0 + ND] = Wfq[:, h * ND:(h + 1) * ND]
            Wqk[:, q0 + ND:q0 + HD] = Wfqr[:, h * RD:(h + 1) * RD]
            Wqk[:, k0:k0 + ND] = Wfk[:, h * ND:(h + 1) * ND]
            Wqk[:, k0 + ND:k0 + HD] = Wk_rope[:, h * RD:(h + 1) * RD]
        Wv = Wfv[:, g * HLOC * HD:(g + 1) * HLOC * HD]
        Wop = Wo[g * HLOC * HD:(g + 1) * HLOC * HD, :]
        in_maps.append({
            "xt": xts[b],
            "wqk": np.ascontiguousarray(
                Wqk.reshape(DT, P, NCT, P).transpose(1, 2, 0, 3)
            ).reshape(P, -1).astype(_BF16),
            "wv": np.ascontiguousarray(
                Wv.reshape(DT, P, HLOC * HD).transpose(1, 0, 2)
            ).reshape(P, -1).astype(_BF16),
            "wo": np.ascontiguousarray(
                Wop.reshape(HLOC, P, D).transpose(1, 0, 2)
            ).reshape(P, -1).astype(_BF16),
            "cosr": cosr,
            "sinr": sinr,
            "maskd": maskd,
        })
    return in_maps


def kernel(x, Wq_down, Wq_up, Wq_rope, Wkv_down, Wk_up, Wk_rope, Wv_up, Wo,
           _trace=False, _trace_kwargs=None):
    x = np.asarray(x, dtype=np.float32)
    args = [np.asarray(a, dtype=np.float32) for a in
            (Wq_down, Wq_up, Wq_rope, Wkv_down, Wk_up, Wk_rope, Wv_up, Wo)]
    in_maps = _host_prep(x, *args)
    nc = _get_nc()
    res = run_bass_kernel_spmd(nc, in_maps, core_ids=list(range(8)),
                               trace=_trace, **(_trace_kwargs or {}))
    kernel._last_results = res
    out = np.zeros((B, S, D), np.float32)
    for c in range(8):
        # un-tile [p, st, d] -> [st*128+p, d]
        part = res.results[c]["o_part"].reshape(P, S // P, D)
        out[c // 4] += part.transpose(1, 0, 2).reshape(S, D).astype(np.float32)
    return out


# revision 23
# speedup vs baseline: 1.2243x; 1.0306x over previous
"""MLA attention (DeepSeek-style) Trainium2 Bass kernel, 8-core SPMD.

Sharding: core c handles batch b = c//4 and head-group g = c%4 (4 of 16 heads).
All low-rank projections are fused on the host (Wq_down@Wq_up etc.), so every
core runs a single head-parallel projection x @ Wqk [D, 1024] (per-head
[q_nope|q_pe] / [k_nope|k_pe] column tiles) + x @ Wv [D, 512] with ZERO
replicated work, then causal flash attention for its 4 heads and a partial
o-projection. Host sums the 4 partial o-projections per batch.

Device dataflow (per core, transposed-activation layout, S processed in 4
chunks of 512):
  xT (host-tiled, bf16) -> per-head qT/kT [128=HD, S] bf16 tiles straight from
  PSUM (nope rows 0:64, rope rows 64:128), RoPE via host-baked cos/sin tables
  -> V in natural layout via swapped-operand matmuls -> causal flash attention
  per head: scoresT [j,i] matmuls with diagonal tiles shrunk to the unmasked
  query range, exp on ScalarE (scale fused) issued one (head,jt) pair ahead of
  its ones/av consumers (software pipelining - no head-boundary PE bubbles),
  unnormalized attnout + ones-matmul row sums, normalize by broadcast
  reciprocal -> o-projection -> partial [S, D] bf16 out (pre-tiled layout).

Every DMA batch gets its own SBUF tile (dependency tracking is per-tile, so
shared tiles would serialize consumers on the LAST dma). ALL loads are issued
up front: mid-compute DMA measurably slows matmuls via SBUF port contention,
so paying ~20us of saturated-DMA startup buys a clean compute window.
"""

import numpy as np
import ml_dtypes

import concourse.bacc as bacc
import concourse.mybir as mybir
import concourse.tile as tile
from concourse.bass_utils import run_bass_kernel_spmd

F32 = mybir.dt.float32
BF16 = mybir.dt.bfloat16

B, S, D = 2, 2048, 2048
H, HD = 16, 128
RD, ND = 64, 64
KVR, QR = 512, 1024
BASE = 10000.0
HLOC = 4                 # heads per core
CHUNK = 512
NCHUNK = S // CHUNK      # 4
P = 128
DT = D // P              # 16 contraction tiles
NCT = 2 * HLOC           # 8 projection c-tiles (4 q heads + 4 k heads)
SCALE = HD ** -0.5

_BF16 = ml_dtypes.bfloat16


def _build():
    nc = bacc.Bacc("TRN2", target_bir_lowering=False, debug=False)

    xt = nc.dram_tensor("xt", [P, NCHUNK * DT * CHUNK], BF16,
                        kind="ExternalInput").ap()
    wqk = nc.dram_tensor("wqk", [P, NCT * DT * P], BF16,
                         kind="ExternalInput").ap()
    wv = nc.dram_tensor("wv", [P, DT * HLOC * HD], BF16,
                        kind="ExternalInput").ap()
    wo = nc.dram_tensor("wo", [P, HLOC * D], BF16, kind="ExternalInput").ap()
    cosr = nc.dram_tensor("cosr", [RD, S], F32, kind="ExternalInput").ap()
    sinr = nc.dram_tensor("sinr", [RD, S], F32, kind="ExternalInput").ap()
    maskd = nc.dram_tensor("maskd", [P, P], BF16, kind="ExternalInput").ap()
    # output pre-tiled [p, st, d]: fat 4KB-per-partition DMA descriptors
    o_part = nc.dram_tensor("o_part", [P, (S // P) * D], BF16,
                            kind="ExternalOutput").ap()

    xt_r = xt.rearrange("p (ic hf dt s) -> p ic hf dt s",
                        ic=NCHUNK, hf=2, dt=DT // 2)
    wqk_r = wqk.rearrange("p (ct dt c) -> p ct dt c", ct=NCT, dt=DT)
    wv_r = wv.rearrange("p (hf dt c) -> p hf dt c", hf=2, dt=DT // 2)
    wo_r = wo.rearrange("p (kt d) -> p kt d", kt=HLOC)
    o_r = o_part.rearrange("p (st d) -> p st d", st=S // P)  # [128, 16, 2048]

    with tile.TileContext(nc) as tc:
        with (
            tc.tile_pool(name="persist", bufs=1) as pp,
            tc.tile_pool(name="acts", bufs=2) as ap_,
            tc.tile_pool(name="rope", bufs=2) as rp,
            tc.tile_pool(name="attn", bufs=3) as atp,
            tc.tile_pool(name="outp", bufs=2) as op_,
            tc.tile_pool(name="aoutp", bufs=2) as aop,
            tc.tile_pool(name="psA", bufs=3, space="PSUM") as psA,
            tc.tile_pool(name="psS", bufs=3, space="PSUM") as psS,
            tc.tile_pool(name="psO", bufs=2, space="PSUM") as psO,
        ):
            # ---------------- persistent tiles (one per DMA batch) ----------
            kTs = [pp.tile([P, S], BF16, tag=f"kT{h}", name=f"kT{h}")
                   for h in range(HLOC)]
            vnat = pp.tile([P, S // P, HLOC * HD], BF16, tag="vnat")
            masks = pp.tile([P, P], BF16, tag="masks")
            ones = pp.tile([P, P], BF16, tag="ones")
            wqkts = [pp.tile([P, DT, P], BF16, tag=f"wqk{ct}", name=f"wqk{ct}")
                     for ct in range(NCT)]
            wvts = [pp.tile([P, DT // 2, HLOC * HD], BF16, tag=f"wv{i}",
                            name=f"wv{i}") for i in range(2)]
            wots = [pp.tile([P, D], BF16, tag=f"wo{kt}", name=f"wo{kt}")
                    for kt in range(HLOC)]
            cos_t = pp.tile([P, S], F32, tag="cos")
            sin_t = pp.tile([P, S], F32, tag="sin")

            nc.vector.memset(ones[:], 1.0)

            def o_proj(ic, aouts):
                for st in range(CHUNK // P):
                    osb = op_.tile([P, D // CHUNK, CHUNK], BF16, tag="osb")
                    for dc in range(D // CHUNK):
                        ps = psA.tile([P, CHUNK], F32, tag="psA")
                        for kt in range(HLOC):
                            nc.tensor.matmul(
                                ps[:], aouts[kt][:, P * st:P * (st + 1)],
                                wots[kt][:, CHUNK * dc:CHUNK * (dc + 1)],
                                start=(kt == 0), stop=(kt == HLOC - 1))
                        if dc % 2 == 0:
                            nc.vector.tensor_copy(osb[:, dc, :], ps[:])
                        else:
                            nc.scalar.copy(osb[:, dc, :], ps[:])
                    nc.scalar.dma_start(
                        o_r[:, ic * (CHUNK // P) + st, :], osb[:])

            def rope_store(ps_pe, dst_pe, cos_c, sin_c):
                """ps_pe: [64, CHUNK] psum AP at partition base 64 (pre-rope pe
                rows of one head). 4 DVE ops; sign-baked sin tables make the
                NeoX rotation a mult/mult/mult/add. dst_pe = rows [64:128]."""
                b = 64
                tmp = rp.tile([P, CHUNK], F32, tag="ropetmp")
                scr = rp.tile([P, CHUNK], F32, tag="ropescr")
                nc.vector.tensor_tensor(tmp[b:b + 32, :], ps_pe[32:64, :],
                                        sin_c[b:b + 32, :], mybir.AluOpType.mult)
                nc.vector.tensor_tensor(tmp[b + 32:b + 64, :], ps_pe[0:32, :],
                                        sin_c[b + 32:b + 64, :],
                                        mybir.AluOpType.mult)
                nc.vector.tensor_tensor(scr[b:b + 64, :], ps_pe[:],
                                        cos_c[b:b + 64, :], mybir.AluOpType.mult)
                nc.vector.tensor_tensor(dst_pe, scr[b:b + 64, :],
                                        tmp[b:b + 64, :], mybir.AluOpType.add)

            # ---------------- chunk loop ----------------
            for ic in range(NCHUNK):
                sl = slice(ic * CHUNK, (ic + 1) * CHUNK)

                xcs = [ap_.tile([P, DT // 2, CHUNK], BF16, tag=f"xc{i}",
                                name=f"xc{i}") for i in range(2)]
                nc.sync.dma_start(xcs[0][:], xt_r[:, ic, 0])
                nc.sync.dma_start(xcs[1][:], xt_r[:, ic, 1])

                def xsl(dt_, cols=slice(None), xcs=xcs):
                    return xcs[dt_ // (DT // 2)][:, dt_ % (DT // 2), cols]

                if ic == 0:
                    for ct in range(NCT):
                        nc.gpsimd.dma_start(wqkts[ct][:], wqk_r[:, ct])
                    nc.scalar.dma_start(cos_t[64:128, :], cosr[:])
                    nc.scalar.dma_start(sin_t[64:128, :], sinr[:])
                    nc.scalar.dma_start(masks[:], maskd[:])
                    nc.gpsimd.dma_start(wvts[0][:], wv_r[:, 0])
                    nc.gpsimd.dma_start(wvts[1][:], wv_r[:, 1])

                cos_c = cos_t[:, sl]
                sin_c = sin_t[:, sl]

                # ---- projection: c-tile ct = head [nope64 | pe64] ----
                # ct 0..3 -> q heads, ct 4..7 -> k heads (identical rope)
                qTis = [ap_.tile([P, CHUNK], BF16, tag=f"qTi{h}",
                                 name=f"qTi{h}") for h in range(HLOC)]
                for ct in range(NCT):
                    ps = psA.tile([P, CHUNK], F32, tag="psA")
                    for dt_ in range(DT):
                        nc.tensor.matmul(
                            ps[:], wqkts[ct][:, dt_, :], xsl(dt_),
                            start=(dt_ == 0), stop=(dt_ == DT - 1))
                    if ct < HLOC:
                        dst_nope = qTis[ct][0:64, :]
                        dst_pe = qTis[ct][64:128, :]
                    else:
                        dst_nope = kTs[ct - HLOC][0:64, sl]
                        dst_pe = kTs[ct - HLOC][64:128, sl]
                    nc.vector.tensor_copy(dst_nope, ps[0:64, :])
                    rope_store(ps[64:128, :], dst_pe, cos_c, sin_c)

                if ic == 0:
                    # resident Wo load, deferred so it doesn't crowd the
                    # critical first-chunk x/weight DMAs
                    for kt in range(HLOC):
                        nc.gpsimd.dma_start(wots[kt][:], wo_r[:, kt])

                # ---- v natural [CHUNK, 512]: x seq-tile stationary ----
                for st in range(CHUNK // P):
                    ps = psA.tile([P, HLOC * HD], F32, tag="psA")
                    for dt_ in range(DT):
                        nc.tensor.matmul(
                            ps[:], xsl(dt_, slice(P * st, P * (st + 1))),
                            wvts[dt_ // (DT // 2)][:, dt_ % (DT // 2), :],
                            start=(dt_ == 0), stop=(dt_ == DT - 1))
                    nc.vector.tensor_copy(vnat[:, ic * (CHUNK // P) + st, :],
                                          ps[:])

                # ---- o-projection of the PREVIOUS chunk: PE work to cover
                # the DVE rope/normalize backlog of this chunk's projections
                if ic > 0:
                    o_proj(ic - 1, prev_aouts)

                # ---- attention for this query chunk ----
                # diagonal j-tiles shrink to queries >= P*r (the rest are
                # fully masked and contribute exact zeros by omission);
                # score+exp issue one (h, jt) pair ahead of ones/av
                aouts = [aop.tile([P, CHUNK], BF16, tag=f"aout{h}",
                                  name=f"aout{h}") for h in range(HLOC)]
                jt_max = (ic + 1) * (CHUNK // P)

                def issue_score(h, jt):
                    r = jt - ic * (CHUNK // P)
                    q0 = P * r if r > 0 else 0
                    pss = psS.tile([P, CHUNK], F32, tag="psS")
                    nc.tensor.matmul(
                        pss[:, q0:], kTs[h][:, P * jt:P * (jt + 1)],
                        qTis[h][:, q0:], start=True, stop=True)
                    at = atp.tile([P, CHUNK], BF16, tag="attnT")
                    nc.scalar.activation(
                        at[:, q0:], pss[:, q0:],
                        mybir.ActivationFunctionType.Exp, scale=SCALE)
                    if r >= 0:  # triangular mask on the diagonal subtile
                        nc.vector.tensor_tensor(
                            at[:, q0:q0 + P], at[:, q0:q0 + P], masks[:],
                            mybir.AluOpType.mult)
                    return at, q0

                pairs = [(h, jt) for h in range(HLOC) for jt in range(jt_max)]
                pending = {pairs[0]: issue_score(*pairs[0])}
                psd = pso = None
                for idx, (h, jt) in enumerate(pairs):
                    if idx + 1 < len(pairs):
                        pending[pairs[idx + 1]] = issue_score(*pairs[idx + 1])
                    at, q0 = pending.pop((h, jt))
                    if jt == 0:
                        psd = psA.tile([P, CHUNK], F32, tag="psA")
                        pso = psO.tile([P, CHUNK], F32, tag="psO")
                    nc.tensor.matmul(psd[:, q0:], ones[:], at[:, q0:],
                                     start=(jt == 0), stop=(jt == jt_max - 1))
                    nc.tensor.matmul(
                        pso[:, q0:], vnat[:, jt, HD * h:HD * (h + 1)],
                        at[:, q0:],
                        start=(jt == 0), stop=(jt == jt_max - 1))
                    if jt == jt_max - 1:
                        rec = atp.tile([P, CHUNK], F32, tag="recip")
                        nc.vector.reciprocal_approx_fast(rec[:], psd[:])
                        nc.vector.tensor_tensor(aouts[h][:], pso[:], rec[:],
                                                mybir.AluOpType.mult)
                prev_aouts = aouts

            o_proj(NCHUNK - 1, prev_aouts)
    nc.compile()
    return nc


_NC = None


def _get_nc():
    global _NC
    if _NC is None:
        _NC = _build()
    return _NC


def _host_prep(x, Wq_down, Wq_up, Wq_rope, Wkv_down, Wk_up, Wk_rope, Wv_up, Wo):
    """Build the 8 per-core input maps (all host-side layout prep)."""
    # rope tables for SBUF partition rows 64:128 (the pe rows), NeoX sign
    # baked into sin
    half = RD // 2
    inv_freq = 1.0 / (BASE ** (np.arange(half, dtype=np.float64) / half))
    ang = np.arange(S, dtype=np.float64)[None, :] * inv_freq[:, None]  # [32, S]
    cos32 = np.cos(ang)
    sin32 = np.sin(ang)
    cosr = np.tile(cos32, (2, 1)).astype(np.float32)                   # [64,S]
    sinr = np.concatenate([-sin32, sin32], 0).astype(np.float32)

    # triangular mask for the 128x128 diagonal subtile: key p <= query i
    pidx = np.arange(P)[:, None]
    iidx = np.arange(P)[None, :]
    maskd = (pidx <= iidx).astype(_BF16)

    # fuse the low-rank compositions once, in f32
    Wfq = Wq_down @ Wq_up        # [D, H*ND]
    Wfqr = Wq_down @ Wq_rope     # [D, H*RD]
    Wfk = Wkv_down @ Wk_up       # [D, H*ND]
    Wfv = Wkv_down @ Wv_up       # [D, H*HD]

    # per-batch pre-tiled x^T: [p, ic, dt, s]
    xts = [np.ascontiguousarray(
        x[b].reshape(NCHUNK, CHUNK, DT, P).transpose(3, 0, 2, 1)
    ).reshape(P, -1).astype(_BF16) for b in range(B)]

    in_maps = []
    for c in range(8):
        b, g = divmod(c, 4)
        heads = range(HLOC * g, HLOC * (g + 1))
        Wqk = np.empty((D, NCT * P), np.float32)
        for i, h in enumerate(heads):
            q0, k0 = i * HD, HLOC * HD + i * HD
            Wqk[:, q0:q0 + ND] = Wfq[:, h * ND:(h + 1) * ND]
            Wqk[:, q0 + ND:q0 + HD] = Wfqr[:, h * RD:(h + 1) * RD]
            Wqk[:, k0:k0 + ND] = Wfk[:, h * ND:(h + 1) * ND]
            Wqk[:, k0 + ND:k0 + HD] = Wk_rope[:, h * RD:(h + 1) * RD]
        Wv = Wfv[:, g * HLOC * HD:(g + 1) * HLOC * HD]
        Wop = Wo[g * HLOC * HD:(g + 1) * HLOC * HD, :]
        in_maps.append({
            "xt": xts[b],
            "wqk": np.ascontiguousarray(
                Wqk.reshape(DT, P, NCT, P).transpose(1, 2, 0, 3)
            ).reshape(P, -1).astype(_BF16),
            "wv": np.ascontiguousarray(
                Wv.reshape(DT, P, HLOC * HD).transpose(1, 0, 2)
            ).reshape(P, -1).astype(_BF16),
            "wo": np.ascontiguousarray(
                Wop.reshape(HLOC, P, D).transpose(1, 0, 2)
            ).reshape(P, -1).astype(_BF16),
            "cosr": cosr,
            "sinr": sinr,
            "maskd": maskd,
        })
    return in_maps


def kernel(x, Wq_down, Wq_up, Wq_rope, Wkv_down, Wk_up, Wk_rope, Wv_up, Wo,
           _trace=False, _trace_kwargs=None):
    x = np.asarray(x, dtype=np.float32)
    args = [np.asarray(a, dtype=np.float32) for a in
            (Wq_down, Wq_up, Wq_rope, Wkv_down, Wk_up, Wk_rope, Wv_up, Wo)]
    in_maps = _host_prep(x, *args)
    nc = _get_nc()
    res = run_bass_kernel_spmd(nc, in_maps, core_ids=list(range(8)),
                               trace=_trace, **(_trace_kwargs or {}))
    kernel._last_results = res
    out = np.zeros((B, S, D), np.float32)
    for c in range(8):
        # un-tile [p, st, d] -> [st*128+p, d]
        part = res.results[c]["o_part"].reshape(P, S // P, D)
        out[c // 4] += part.transpose(1, 0, 2).reshape(S, D).astype(np.float32)
    return out


# revision 24
# speedup vs baseline: 1.2308x; 1.0054x over previous
"""MLA attention (DeepSeek-style) Trainium2 Bass kernel, 8-core SPMD.

Sharding: core c handles batch b = c//4 and head-group g = c%4 (4 of 16 heads).
All low-rank projections are fused on the host (Wq_down@Wq_up etc.), so every
core runs a single head-parallel projection x @ Wqk [D, 1024] (per-head
[q_nope|q_pe] / [k_nope|k_pe] column tiles) + x @ Wv [D, 512] with ZERO
replicated work, then causal flash attention for its 4 heads and a partial
o-projection. Host sums the 4 partial o-projections per batch.

Device dataflow (per core, transposed-activation layout, S processed in 4
chunks of 512):
  xT (host-tiled, bf16) -> per-head qT/kT [128=HD, S] bf16 tiles straight from
  PSUM (nope rows 0:64, rope rows 64:128), RoPE via host-baked cos/sin tables
  -> V in natural layout via swapped-operand matmuls -> causal flash attention
  per head: scoresT [j,i] matmuls with diagonal tiles shrunk to the unmasked
  query range, exp on ScalarE (scale fused) issued one (head,jt) pair ahead of
  its ones/av consumers (software pipelining - no head-boundary PE bubbles),
  unnormalized attnout + ones-matmul row sums, normalize by broadcast
  reciprocal -> o-projection -> partial [S, D] bf16 out (pre-tiled layout).

Every DMA batch gets its own SBUF tile (dependency tracking is per-tile, so
shared tiles would serialize consumers on the LAST dma). ALL loads are issued
up front: mid-compute DMA measurably slows matmuls via SBUF port contention,
so paying ~20us of saturated-DMA startup buys a clean compute window.
"""

import numpy as np
import ml_dtypes

import concourse.bacc as bacc
import concourse.mybir as mybir
import concourse.tile as tile
from concourse.bass_utils import run_bass_kernel_spmd

F32 = mybir.dt.float32
BF16 = mybir.dt.bfloat16

B, S, D = 2, 2048, 2048
H, HD = 16, 128
RD, ND = 64, 64
KVR, QR = 512, 1024
BASE = 10000.0
HLOC = 4                 # heads per core
CHUNK = 512
NCHUNK = S // CHUNK      # 4
P = 128
DT = D // P              # 16 contraction tiles
NCT = 2 * HLOC           # 8 projection c-tiles (4 q heads + 4 k heads)
SCALE = HD ** -0.5

_BF16 = ml_dtypes.bfloat16


def _build():
    nc = bacc.Bacc("TRN2", target_bir_lowering=False, debug=False)

    xt = nc.dram_tensor("xt", [P, NCHUNK * DT * CHUNK], BF16,
                        kind="ExternalInput").ap()
    wqk = nc.dram_tensor("wqk", [P, NCT * DT * P], BF16,
                         kind="ExternalInput").ap()
    wv = nc.dram_tensor("wv", [P, DT * HLOC * HD], BF16,
                        kind="ExternalInput").ap()
    wo = nc.dram_tensor("wo", [P, HLOC * D], BF16, kind="ExternalInput").ap()
    cosr = nc.dram_tensor("cosr", [RD, S], F32, kind="ExternalInput").ap()
    sinr = nc.dram_tensor("sinr", [RD, S], F32, kind="ExternalInput").ap()
    maskd = nc.dram_tensor("maskd", [P, P], BF16, kind="ExternalInput").ap()
    # output pre-tiled [p, st, d]: fat 4KB-per-partition DMA descriptors
    o_part = nc.dram_tensor("o_part", [P, (S // P) * D], BF16,
                            kind="ExternalOutput").ap()

    xt_r = xt.rearrange("p (ic hf dt s) -> p ic hf dt s",
                        ic=NCHUNK, hf=2, dt=DT // 2)
    wqk_r = wqk.rearrange("p (ct dt c) -> p ct dt c", ct=NCT, dt=DT)
    wv_r = wv.rearrange("p (hf dt c) -> p hf dt c", hf=2, dt=DT // 2)
    wo_r = wo.rearrange("p (kt d) -> p kt d", kt=HLOC)
    o_r = o_part.rearrange("p (st d) -> p st d", st=S // P)  # [128, 16, 2048]

    with tile.TileContext(nc) as tc:
        with (
            tc.tile_pool(name="persist", bufs=1) as pp,
            tc.tile_pool(name="acts", bufs=2) as ap_,
            tc.tile_pool(name="rope", bufs=2) as rp,
            tc.tile_pool(name="attn", bufs=3) as atp,
            tc.tile_pool(name="outp", bufs=2) as op_,
            tc.tile_pool(name="aoutp", bufs=2) as aop,
            tc.tile_pool(name="psA", bufs=3, space="PSUM") as psA,
            tc.tile_pool(name="psS", bufs=3, space="PSUM") as psS,
            tc.tile_pool(name="psO", bufs=2, space="PSUM") as psO,
        ):
            # ---------------- persistent tiles (one per DMA batch) ----------
            kTs = [pp.tile([P, S], BF16, tag=f"kT{h}", name=f"kT{h}")
                   for h in range(HLOC)]
            vnat = pp.tile([P, S // P, HLOC * HD], BF16, tag="vnat")
            masks = pp.tile([P, P], BF16, tag="masks")
            ones = pp.tile([P, P], BF16, tag="ones")
            wqkts = [pp.tile([P, DT, P], BF16, tag=f"wqk{ct}", name=f"wqk{ct}")
                     for ct in range(NCT)]
            wvts = [pp.tile([P, DT // 2, HLOC * HD], BF16, tag=f"wv{i}",
                            name=f"wv{i}") for i in range(2)]
            wots = [pp.tile([P, D], BF16, tag=f"wo{kt}", name=f"wo{kt}")
                    for kt in range(HLOC)]
            cos_t = pp.tile([P, S], F32, tag="cos")
            sin_t = pp.tile([P, S], F32, tag="sin")
            dummy = pp.tile([P, 4], BF16, tag="dummy")

            nc.vector.memset(ones[:], 1.0)

            def o_proj(ic, aouts):
                for st in range(CHUNK // P):
                    osb = op_.tile([P, D // CHUNK, CHUNK], BF16, tag="osb")
                    for dc in range(D // CHUNK):
                        ps = psA.tile([P, CHUNK], F32, tag="psA")
                        for kt in range(HLOC):
                            nc.tensor.matmul(
                                ps[:], aouts[kt][:, P * st:P * (st + 1)],
                                wots[kt][:, CHUNK * dc:CHUNK * (dc + 1)],
                                start=(kt == 0), stop=(kt == HLOC - 1))
                        if dc % 2 == 0:
                            nc.vector.tensor_copy(osb[:, dc, :], ps[:])
                        else:
                            nc.scalar.copy(osb[:, dc, :], ps[:])
                    nc.scalar.dma_start(
                        o_r[:, ic * (CHUNK // P) + st, :], osb[:])

            def rope_store(ps_pe, dst_pe, cos_c, sin_c):
                """ps_pe: [64, CHUNK] psum AP at partition base 64 (pre-rope pe
                rows of one head). 4 DVE ops; sign-baked sin tables make the
                NeoX rotation a mult/mult/mult/add. dst_pe = rows [64:128]."""
                b = 64
                tmp = rp.tile([P, CHUNK], F32, tag="ropetmp")
                scr = rp.tile([P, CHUNK], F32, tag="ropescr")
                nc.vector.tensor_tensor(tmp[b:b + 32, :], ps_pe[32:64, :],
                                        sin_c[b:b + 32, :], mybir.AluOpType.mult)
                nc.vector.tensor_tensor(tmp[b + 32:b + 64, :], ps_pe[0:32, :],
                                        sin_c[b + 32:b + 64, :],
                                        mybir.AluOpType.mult)
                nc.vector.tensor_tensor(scr[b:b + 64, :], ps_pe[:],
                                        cos_c[b:b + 64, :], mybir.AluOpType.mult)
                nc.vector.tensor_tensor(dst_pe, scr[b:b + 64, :],
                                        tmp[b:b + 64, :], mybir.AluOpType.add)

            # ---------------- chunk loop ----------------
            for ic in range(NCHUNK):
                sl = slice(ic * CHUNK, (ic + 1) * CHUNK)

                if ic == 0:
                    xcs = [ap_.tile([P, DT // 2, CHUNK], BF16, tag=f"xc{i}",
                                    name=f"xc{i}") for i in range(2)]
                    nc.sync.dma_start(xcs[0][:], xt_r[:, ic, 0])
                    nc.sync.dma_start(xcs[1][:], xt_r[:, ic, 1])
                    next_xcs = [ap_.tile([P, DT // 2, CHUNK], BF16,
                                         tag=f"xc{i}", name=f"xc{i}")
                                for i in range(2)]
                else:
                    xcs = cur_xcs
                if 1 <= ic < NCHUNK - 1:
                    next_xcs = [ap_.tile([P, DT // 2, CHUNK], BF16,
                                         tag=f"xc{i}", name=f"xc{i}")
                                for i in range(2)]
                    nc.sync.dma_start(next_xcs[0][:], xt_r[:, ic + 1, 0])
                    nc.sync.dma_start(next_xcs[1][:], xt_r[:, ic + 1, 1])
                cur_xcs = next_xcs

                def xsl(dt_, cols=slice(None), xcs=xcs):
                    return xcs[dt_ // (DT // 2)][:, dt_ % (DT // 2), cols]

                if ic == 0:
                    # critical set races exclusively: xc0/xc1 (sync ring),
                    # wqk strips 0-1 (gpsimd ring), rope tables (scalar ring)
                    nc.gpsimd.dma_start(wqkts[0][:], wqk_r[:, 0])
                    nc.gpsimd.dma_start(wqkts[1][:], wqk_r[:, 1])
                    nc.scalar.dma_start(cos_t[64:128, :], cosr[:])
                    nc.scalar.dma_start(sin_t[64:128, :], sinr[:])
                    nc.scalar.dma_start(masks[:], maskd[:])
                    # bulk (9MB) gated behind xc0 arrival via a dummy copy:
                    # it cannot steal SDMA packet slots from the critical set
                    nc.scalar.copy(dummy[0:1, 0:1], xcs[0][0:1, 0, 0:1])
                    for c2 in range(2, NCT):
                        nc.scalar.dma_start(wqkts[c2][:], wqk_r[:, c2])
                    nc.scalar.dma_start(wvts[0][:], wv_r[:, 0])
                    nc.scalar.dma_start(wvts[1][:], wv_r[:, 1])

                cos_c = cos_t[:, sl]
                sin_c = sin_t[:, sl]

                # ---- projection: c-tile ct = head [nope64 | pe64] ----
                # ct 0..3 -> q heads, ct 4..7 -> k heads (identical rope)
                qTis = [ap_.tile([P, CHUNK], BF16, tag=f"qTi{h}",
                                 name=f"qTi{h}") for h in range(HLOC)]
                for ct in range(NCT):
                    ps = psA.tile([P, CHUNK], F32, tag="psA")
                    for dt_ in range(DT):
                        nc.tensor.matmul(
                            ps[:], wqkts[ct][:, dt_, :], xsl(dt_),
                            start=(dt_ == 0), stop=(dt_ == DT - 1))
                    if ct < HLOC:
                        dst_nope = qTis[ct][0:64, :]
                        dst_pe = qTis[ct][64:128, :]
                    else:
                        dst_nope = kTs[ct - HLOC][0:64, sl]
                        dst_pe = kTs[ct - HLOC][64:128, sl]
                    nc.vector.tensor_copy(dst_nope, ps[0:64, :])
                    rope_store(ps[64:128, :], dst_pe, cos_c, sin_c)

                if ic == 0:
                    # Wo + chunk-1 x, tail of the gated bulk (scalar FIFO)
                    nc.scalar.dma_start(next_xcs[0][:], xt_r[:, 1, 0])
                    nc.scalar.dma_start(next_xcs[1][:], xt_r[:, 1, 1])
                    for kt in range(HLOC):
                        nc.scalar.dma_start(wots[kt][:], wo_r[:, kt])

                # ---- v natural [CHUNK, 512]: x seq-tile stationary ----
                for st in range(CHUNK // P):
                    ps = psA.tile([P, HLOC * HD], F32, tag="psA")
                    for dt_ in range(DT):
                        nc.tensor.matmul(
                            ps[:], xsl(dt_, slice(P * st, P * (st + 1))),
                            wvts[dt_ // (DT // 2)][:, dt_ % (DT // 2), :],
                            start=(dt_ == 0), stop=(dt_ == DT - 1))
                    nc.vector.tensor_copy(vnat[:, ic * (CHUNK // P) + st, :],
                                          ps[:])

                # ---- o-projection of the PREVIOUS chunk: PE work to cover
                # the DVE rope/normalize backlog of this chunk's projections
                if ic > 0:
                    o_proj(ic - 1, prev_aouts)

                # ---- attention for this query chunk ----
                # diagonal j-tiles shrink to queries >= P*r (the rest are
                # fully masked and contribute exact zeros by omission);
                # score+exp issue one (h, jt) pair ahead of ones/av
                aouts = [aop.tile([P, CHUNK], BF16, tag=f"aout{h}",
                                  name=f"aout{h}") for h in range(HLOC)]
                jt_max = (ic + 1) * (CHUNK // P)

                def issue_score(h, jt):
                    r = jt - ic * (CHUNK // P)
                    q0 = P * r if r > 0 else 0
                    pss = psS.tile([P, CHUNK], F32, tag="psS")
                    nc.tensor.matmul(
                        pss[:, q0:], kTs[h][:, P * jt:P * (jt + 1)],
                        qTis[h][:, q0:], start=True, stop=True)
                    at = atp.tile([P, CHUNK], BF16, tag="attnT")
                    nc.scalar.activation(
                        at[:, q0:], pss[:, q0:],
                        mybir.ActivationFunctionType.Exp, scale=SCALE)
                    if r >= 0:  # triangular mask on the diagonal subtile
                        nc.vector.tensor_tensor(
                            at[:, q0:q0 + P], at[:, q0:q0 + P], masks[:],
                            mybir.AluOpType.mult)
                    return at, q0

                pairs = [(h, jt) for h in range(HLOC) for jt in range(jt_max)]
                pending = {pairs[0]: issue_score(*pairs[0])}
                psd = pso = None
                for idx, (h, jt) in enumerate(pairs):
                    if idx + 1 < len(pairs):
                        pending[pairs[idx + 1]] = issue_score(*pairs[idx + 1])
                    at, q0 = pending.pop((h, jt))
                    if jt == 0:
                        psd = psA.tile([P, CHUNK], F32, tag="psA")
                        pso = psO.tile([P, CHUNK], F32, tag="psO")
                    nc.tensor.matmul(psd[:, q0:], ones[:], at[:, q0:],
                                     start=(jt == 0), stop=(jt == jt_max - 1))
                    nc.tensor.matmul(
                        pso[:, q0:], vnat[:, jt, HD * h:HD * (h + 1)],
                        at[:, q0:],
                        start=(jt == 0), stop=(jt == jt_max - 1))
                    if jt == jt_max - 1:
                        rec = atp.tile([P, CHUNK], F32, tag="recip")
                        nc.vector.reciprocal_approx_fast(rec[:], psd[:])
                        nc.vector.tensor_tensor(aouts[h][:], pso[:], rec[:],
                                                mybir.AluOpType.mult)
                prev_aouts = aouts

            o_proj(NCHUNK - 1, prev_aouts)
    nc.compile()
    return nc


_NC = None


def _get_nc():
    global _NC
    if _NC is None:
        _NC = _build()
    return _NC


def _host_prep(x, Wq_down, Wq_up, Wq_rope, Wkv_down, Wk_up, Wk_rope, Wv_up, Wo):
    """Build the 8 per-core input maps (all host-side layout prep)."""
    # rope tables for SBUF partition rows 64:128 (the pe rows), NeoX sign
    # baked into sin
    half = RD // 2
    inv_freq = 1.0 / (BASE ** (np.arange(half, dtype=np.float64) / half))
    ang = np.arange(S, dtype=np.float64)[None, :] * inv_freq[:, None]  # [32, S]
    cos32 = np.cos(ang)
    sin32 = np.sin(ang)
    cosr = np.tile(cos32, (2, 1)).astype(np.float32)                   # [64,S]
    sinr = np.concatenate([-sin32, sin32], 0).astype(np.float32)

    # triangular mask for the 128x128 diagonal subtile: key p <= query i
    pidx = np.arange(P)[:, None]
    iidx = np.arange(P)[None, :]
    maskd = (pidx <= iidx).astype(_BF16)

    # fuse the low-rank compositions once, in f32
    Wfq = Wq_down @ Wq_up        # [D, H*ND]
    Wfqr = Wq_down @ Wq_rope     # [D, H*RD]
    Wfk = Wkv_down @ Wk_up       # [D, H*ND]
    Wfv = Wkv_down @ Wv_up       # [D, H*HD]

    # per-batch pre-tiled x^T: [p, ic, dt, s]
    xts = [np.ascontiguousarray(
        x[b].reshape(NCHUNK, CHUNK, DT, P).transpose(3, 0, 2, 1)
    ).reshape(P, -1).astype(_BF16) for b in range(B)]

    in_maps = []
    for c in range(8):
        b, g = divmod(c, 4)
        heads = range(HLOC * g, HLOC * (g + 1))
        Wqk = np.empty((D, NCT * P), np.float32)
        for i, h in enumerate(heads):
            q0, k0 = i * HD, HLOC * HD + i * HD
            Wqk[:, q0:q0 + ND] = Wfq[:, h * ND:(h + 1) * ND]
            Wqk[:, q0 + ND:q0 + HD] = Wfqr[:, h * RD:(h + 1) * RD]
            Wqk[:, k0:k0 + ND] = Wfk[:, h * ND:(h + 1) * ND]
            Wqk[:, k0 + ND:k0 + HD] = Wk_rope[:, h * RD:(h + 1) * RD]
        Wv = Wfv[:, g * HLOC * HD:(g + 1) * HLOC * HD]
        Wop = Wo[g * HLOC * HD:(g + 1) * HLOC * HD, :]
        in_maps.append({
            "xt": xts[b],
            "wqk": np.ascontiguousarray(
                Wqk.reshape(DT, P, NCT, P).transpose(1, 2, 0, 3)
            ).reshape(P, -1).astype(_BF16),
            "wv": np.ascontiguousarray(
                Wv.reshape(DT, P, HLOC * HD).transpose(1, 0, 2)
            ).reshape(P, -1).astype(_BF16),
            "wo": np.ascontiguousarray(
                Wop.reshape(HLOC, P, D).transpose(1, 0, 2)
            ).reshape(P, -1).astype(_BF16),
            "cosr": cosr,
            "sinr": sinr,
            "maskd": maskd,
        })
    return in_maps


def kernel(x, Wq_down, Wq_up, Wq_rope, Wkv_down, Wk_up, Wk_rope, Wv_up, Wo,
           _trace=False, _trace_kwargs=None):
    x = np.asarray(x, dtype=np.float32)
    args = [np.asarray(a, dtype=np.float32) for a in
            (Wq_down, Wq_up, Wq_rope, Wkv_down, Wk_up, Wk_rope, Wv_up, Wo)]
    in_maps = _host_prep(x, *args)
    nc = _get_nc()
    res = run_bass_kernel_spmd(nc, in_maps, core_ids=list(range(8)),
                               trace=_trace, **(_trace_kwargs or {}))
    kernel._last_results = res
    out = np.zeros((B, S, D), np.float32)
    for c in range(8):
        # un-tile [p, st, d] -> [st*128+p, d]
        part = res.results[c]["o_part"].reshape(P, S // P, D)
        out[c // 4] += part.transpose(1, 0, 2).reshape(S, D).astype(np.float32)
    return out


# revision 25
# speedup vs baseline: 1.2335x; 1.0022x over previous
"""MLA attention (DeepSeek-style) Trainium2 Bass kernel, 8-core SPMD.

Sharding: core c handles batch b = c//4 and head-group g = c%4 (4 of 16 heads).
All low-rank projections are fused on the host (Wq_down@Wq_up etc.), so every
core runs a single head-parallel projection x @ Wqk [D, 1024] (per-head
[q_nope|q_pe] / [k_nope|k_pe] column tiles) + x @ Wv [D, 512] with ZERO
replicated work, then causal flash attention for its 4 heads and a partial
o-projection. Host sums the 4 partial o-projections per batch.

Device dataflow (per core, transposed-activation layout, S processed in 4
chunks of 512):
  xT (host-tiled, bf16) -> per-head qT/kT [128=HD, S] bf16 tiles straight from
  PSUM (nope rows 0:64, rope rows 64:128), RoPE via host-baked cos/sin tables
  -> V in natural layout via swapped-operand matmuls -> causal flash attention
  per head: scoresT [j,i] matmuls with diagonal tiles shrunk to the unmasked
  query range, exp on ScalarE (scale fused) issued one (head,jt) pair ahead of
  its ones/av consumers (software pipelining - no head-boundary PE bubbles),
  unnormalized attnout + ones-matmul row sums, normalize by broadcast
  reciprocal -> o-projection -> partial [S, D] bf16 out (pre-tiled layout).

Every DMA batch gets its own SBUF tile (dependency tracking is per-tile, so
shared tiles would serialize consumers on the LAST dma). ALL loads are issued
up front: mid-compute DMA measurably slows matmuls via SBUF port contention,
so paying ~20us of saturated-DMA startup buys a clean compute window.
"""

import numpy as np
import ml_dtypes

import concourse.bacc as bacc
import concourse.mybir as mybir
import concourse.tile as tile
from concourse.bass_utils import run_bass_kernel_spmd

F32 = mybir.dt.float32
BF16 = mybir.dt.bfloat16

B, S, D = 2, 2048, 2048
H, HD = 16, 128
RD, ND = 64, 64
KVR, QR = 512, 1024
BASE = 10000.0
HLOC = 4                 # heads per core
CHUNK = 512
NCHUNK = S // CHUNK      # 4
P = 128
DT = D // P              # 16 contraction tiles
NCT = 2 * HLOC           # 8 projection c-tiles (4 q heads + 4 k heads)
SCALE = HD ** -0.5

_BF16 = ml_dtypes.bfloat16


def _build():
    nc = bacc.Bacc("TRN2", target_bir_lowering=False, debug=False)

    xt = nc.dram_tensor("xt", [P, NCHUNK * DT * CHUNK], BF16,
                        kind="ExternalInput").ap()
    wqk = nc.dram_tensor("wqk", [P, NCT * DT * P], BF16,
                         kind="ExternalInput").ap()
    wv = nc.dram_tensor("wv", [P, DT * HLOC * HD], BF16,
                        kind="ExternalInput").ap()
    wo = nc.dram_tensor("wo", [P, HLOC * D], BF16, kind="ExternalInput").ap()
    cosr = nc.dram_tensor("cosr", [RD, S], F32, kind="ExternalInput").ap()
    sinr = nc.dram_tensor("sinr", [RD, S], F32, kind="ExternalInput").ap()
    maskd = nc.dram_tensor("maskd", [P, P], BF16, kind="ExternalInput").ap()
    # output pre-tiled [p, st, d]: fat 4KB-per-partition DMA descriptors
    o_part = nc.dram_tensor("o_part", [P, (S // P) * D], BF16,
                            kind="ExternalOutput").ap()

    xt_r = xt.rearrange("p (ic hf dt s) -> p ic hf dt s",
                        ic=NCHUNK, hf=2, dt=DT // 2)
    xt_r2 = xt.rearrange("p (ic qf dt s) -> p ic qf dt s",
                         ic=NCHUNK, qf=4, dt=DT // 4)
    wqk_r = wqk.rearrange("p (ct dt c) -> p ct dt c", ct=NCT, dt=DT)
    wv_r = wv.rearrange("p (hf dt c) -> p hf dt c", hf=2, dt=DT // 2)
    wo_r = wo.rearrange("p (kt d) -> p kt d", kt=HLOC)
    o_r = o_part.rearrange("p (st d) -> p st d", st=S // P)  # [128, 16, 2048]

    with tile.TileContext(nc) as tc:
        with (
            tc.tile_pool(name="persist", bufs=1) as pp,
            tc.tile_pool(name="acts", bufs=2) as ap_,
            tc.tile_pool(name="rope", bufs=2) as rp,
            tc.tile_pool(name="attn", bufs=3) as atp,
            tc.tile_pool(name="outp", bufs=2) as op_,
            tc.tile_pool(name="aoutp", bufs=2) as aop,
            tc.tile_pool(name="psA", bufs=3, space="PSUM") as psA,
            tc.tile_pool(name="psS", bufs=3, space="PSUM") as psS,
            tc.tile_pool(name="psO", bufs=2, space="PSUM") as psO,
        ):
            # ---------------- persistent tiles (one per DMA batch) ----------
            kTs = [pp.tile([P, S], BF16, tag=f"kT{h}", name=f"kT{h}")
                   for h in range(HLOC)]
            vnat = pp.tile([P, S // P, HLOC * HD], BF16, tag="vnat")
            masks = pp.tile([P, P], BF16, tag="masks")
            ones = pp.tile([P, P], BF16, tag="ones")
            wqkts = [pp.tile([P, DT, P], BF16, tag=f"wqk{ct}", name=f"wqk{ct}")
                     for ct in range(NCT)]
            wvts = [pp.tile([P, DT // 2, HLOC * HD], BF16, tag=f"wv{i}",
                            name=f"wv{i}") for i in range(2)]
            wots = [pp.tile([P, D], BF16, tag=f"wo{kt}", name=f"wo{kt}")
                    for kt in range(HLOC)]
            cos_t = pp.tile([P, S], F32, tag="cos")
            sin_t = pp.tile([P, S], F32, tag="sin")
            dummy = pp.tile([P, 4], BF16, tag="dummy")

            nc.vector.memset(ones[:], 1.0)

            def o_proj(ic, aouts):
                for st in range(CHUNK // P):
                    osb = op_.tile([P, D // CHUNK, CHUNK], BF16, tag="osb")
                    for dc in range(D // CHUNK):
                        ps = psA.tile([P, CHUNK], F32, tag="psA")
                        for kt in range(HLOC):
                            nc.tensor.matmul(
                                ps[:], aouts[kt][:, P * st:P * (st + 1)],
                                wots[kt][:, CHUNK * dc:CHUNK * (dc + 1)],
                                start=(kt == 0), stop=(kt == HLOC - 1))
                        if dc % 2 == 0:
                            nc.scalar.copy(osb[:, dc, :], ps[:])
                        else:
                            nc.vector.tensor_copy(osb[:, dc, :], ps[:])
                    nc.scalar.dma_start(
                        o_r[:, ic * (CHUNK // P) + st, :], osb[:])

            def rope_store(ps_pe, dst_pe, cos_c, sin_c):
                """ps_pe: [64, CHUNK] psum AP at partition base 64 (pre-rope pe
                rows of one head). 4 DVE ops; sign-baked sin tables make the
                NeoX rotation a mult/mult/mult/add. dst_pe = rows [64:128]."""
                b = 64
                tmp = rp.tile([P, CHUNK], F32, tag="ropetmp")
                scr = rp.tile([P, CHUNK], F32, tag="ropescr")
                nc.vector.tensor_tensor(tmp[b:b + 32, :], ps_pe[32:64, :],
                                        sin_c[b:b + 32, :], mybir.AluOpType.mult)
                nc.vector.tensor_tensor(tmp[b + 32:b + 64, :], ps_pe[0:32, :],
                                        sin_c[b + 32:b + 64, :],
                                        mybir.AluOpType.mult)
                nc.vector.tensor_tensor(scr[b:b + 64, :], ps_pe[:],
                                        cos_c[b:b + 64, :], mybir.AluOpType.mult)
                nc.vector.tensor_tensor(dst_pe, scr[b:b + 64, :],
                                        tmp[b:b + 64, :], mybir.AluOpType.add)

            # ---------------- chunk loop ----------------
            for ic in range(NCHUNK):
                sl = slice(ic * CHUNK, (ic + 1) * CHUNK)

                if ic == 0:
                    xcs = [ap_.tile([P, DT // 4, CHUNK], BF16, tag=f"xc{i}",
                                    name=f"xc{i}") for i in range(4)]
                    for i in range(4):
                        nc.sync.dma_start(xcs[i][:], xt_r2[:, ic, i])
                    next_xcs = [ap_.tile([P, DT // 4, CHUNK], BF16,
                                         tag=f"xc{i}", name=f"xc{i}")
                                for i in range(4)]
                else:
                    xcs = cur_xcs
                if 1 <= ic < NCHUNK - 1:
                    next_xcs = [ap_.tile([P, DT // 4, CHUNK], BF16,
                                         tag=f"xc{i}", name=f"xc{i}")
                                for i in range(4)]
                    for i in range(4):
                        nc.sync.dma_start(next_xcs[i][:], xt_r2[:, ic + 1, i])
                cur_xcs = next_xcs

                def xsl(dt_, cols=slice(None), xcs=xcs):
                    return xcs[dt_ // (DT // 4)][:, dt_ % (DT // 4), cols]

                if ic == 0:
                    # critical set races exclusively: xc0/xc1 (sync ring),
                    # wqk strips 0-1 (gpsimd ring), rope tables (scalar ring)
                    nc.gpsimd.dma_start(wqkts[0][:], wqk_r[:, 0])
                    nc.gpsimd.dma_start(wqkts[1][:], wqk_r[:, 1])
                    nc.scalar.dma_start(cos_t[64:128, :], cosr[:])
                    nc.scalar.dma_start(sin_t[64:128, :], sinr[:])
                    nc.scalar.dma_start(masks[:], maskd[:])
                    # bulk (9MB) gated behind xc0 arrival via a dummy copy:
                    # it cannot steal SDMA packet slots from the critical set
                    nc.scalar.copy(dummy[0:1, 0:1], xcs[0][0:1, 0, 0:1])
                    for c2 in range(2, NCT):
                        nc.scalar.dma_start(wqkts[c2][:], wqk_r[:, c2])
                    nc.scalar.dma_start(wvts[0][:], wv_r[:, 0])
                    nc.scalar.dma_start(wvts[1][:], wv_r[:, 1])

                cos_c = cos_t[:, sl]
                sin_c = sin_t[:, sl]

                # ---- projection: c-tile ct = head [nope64 | pe64] ----
                # ct 0..3 -> q heads, ct 4..7 -> k heads (identical rope)
                qTis = [ap_.tile([P, CHUNK], BF16, tag=f"qTi{h}",
                                 name=f"qTi{h}") for h in range(HLOC)]
                for ct in range(NCT):
                    ps = psA.tile([P, CHUNK], F32, tag="psA")
                    for dt_ in range(DT):
                        nc.tensor.matmul(
                            ps[:], wqkts[ct][:, dt_, :], xsl(dt_),
                            start=(dt_ == 0), stop=(dt_ == DT - 1))
                    if ct < HLOC:
                        dst_nope = qTis[ct][0:64, :]
                        dst_pe = qTis[ct][64:128, :]
                    else:
                        dst_nope = kTs[ct - HLOC][0:64, sl]
                        dst_pe = kTs[ct - HLOC][64:128, sl]
                    nc.vector.tensor_copy(dst_nope, ps[0:64, :])
                    rope_store(ps[64:128, :], dst_pe, cos_c, sin_c)

                if ic == 0:
                    # Wo + chunk-1 x, tail of the gated bulk (scalar FIFO)
                    for i in range(4):
                        nc.scalar.dma_start(next_xcs[i][:], xt_r2[:, 1, i])
                    for kt in range(HLOC):
                        nc.scalar.dma_start(wots[kt][:], wo_r[:, kt])

                # ---- v natural [CHUNK, 512]: x seq-tile stationary ----
                for st in range(CHUNK // P):
                    ps = psA.tile([P, HLOC * HD], F32, tag="psA")
                    for dt_ in range(DT):
                        nc.tensor.matmul(
                            ps[:], xsl(dt_, slice(P * st, P * (st + 1))),
                            wvts[dt_ // (DT // 2)][:, dt_ % (DT // 2), :],
                            start=(dt_ == 0), stop=(dt_ == DT - 1))
                    nc.vector.tensor_copy(vnat[:, ic * (CHUNK // P) + st, :],
                                          ps[:])

                # ---- o-projection of the PREVIOUS chunk: PE work to cover
                # the DVE rope/normalize backlog of this chunk's projections
                if ic > 0:
                    o_proj(ic - 1, prev_aouts)

                # ---- attention for this query chunk ----
                # diagonal j-tiles shrink to queries >= P*r (the rest are
                # fully masked and contribute exact zeros by omission);
                # score+exp issue one (h, jt) pair ahead of ones/av
                aouts = [aop.tile([P, CHUNK], BF16, tag=f"aout{h}",
                                  name=f"aout{h}") for h in range(HLOC)]
                jt_max = (ic + 1) * (CHUNK // P)

                def issue_score(h, jt):
                    r = jt - ic * (CHUNK // P)
                    q0 = P * r if r > 0 else 0
                    pss = psS.tile([P, CHUNK], F32, tag="psS")
                    nc.tensor.matmul(
                        pss[:, q0:], kTs[h][:, P * jt:P * (jt + 1)],
                        qTis[h][:, q0:], start=True, stop=True)
                    at = atp.tile([P, CHUNK], BF16, tag="attnT")
                    nc.scalar.activation(
                        at[:, q0:], pss[:, q0:],
                        mybir.ActivationFunctionType.Exp, scale=SCALE)
                    if r >= 0:  # triangular mask on the diagonal subtile
                        nc.vector.tensor_tensor(
                            at[:, q0:q0 + P], at[:, q0:q0 + P], masks[:],
                            mybir.AluOpType.mult)
                    return at, q0

                pairs = [(h, jt) for h in range(HLOC) for jt in range(jt_max)]
                pending = {pairs[0]: issue_score(*pairs[0])}
                psd = pso = None
                for idx, (h, jt) in enumerate(pairs):
                    if idx + 1 < len(pairs):
                        pending[pairs[idx + 1]] = issue_score(*pairs[idx + 1])
                    at, q0 = pending.pop((h, jt))
                    if jt == 0:
                        psd = psA.tile([P, CHUNK], F32, tag="psA")
                        pso = psO.tile([P, CHUNK], F32, tag="psO")
                    nc.tensor.matmul(psd[:, q0:], ones[:], at[:, q0:],
                                     start=(jt == 0), stop=(jt == jt_max - 1))
                    nc.tensor.matmul(
                        pso[:, q0:], vnat[:, jt, HD * h:HD * (h + 1)],
                        at[:, q0:],
                        start=(jt == 0), stop=(jt == jt_max - 1))
                    if jt == jt_max - 1:
                        rec = atp.tile([P, CHUNK], F32, tag="recip")
                        nc.vector.reciprocal_approx_fast(rec[:], psd[:])
                        nc.vector.tensor_tensor(aouts[h][:], pso[:], rec[:],
                                                mybir.AluOpType.mult)
                prev_aouts = aouts

            o_proj(NCHUNK - 1, prev_aouts)
    nc.compile()
    return nc


_NC = None


def _get_nc():
    global _NC
    if _NC is None:
        _NC = _build()
    return _NC


def _host_prep(x, Wq_down, Wq_up, Wq_rope, Wkv_down, Wk_up, Wk_rope, Wv_up, Wo):
    """Build the 8 per-core input maps (all host-side layout prep)."""
    # rope tables for SBUF partition rows 64:128 (the pe rows), NeoX sign
    # baked into sin
    half = RD // 2
    inv_freq = 1.0 / (BASE ** (np.arange(half, dtype=np.float64) / half))
    ang = np.arange(S, dtype=np.float64)[None, :] * inv_freq[:, None]  # [32, S]
    cos32 = np.cos(ang)
    sin32 = np.sin(ang)
    cosr = np.tile(cos32, (2, 1)).astype(np.float32)                   # [64,S]
    sinr = np.concatenate([-sin32, sin32], 0).astype(np.float32)

    # triangular mask for the 128x128 diagonal subtile: key p <= query i
    pidx = np.arange(P)[:, None]
    iidx = np.arange(P)[None, :]
    maskd = (pidx <= iidx).astype(_BF16)

    # fuse the low-rank compositions once, in f32
    Wfq = Wq_down @ Wq_up        # [D, H*ND]
    Wfqr = Wq_down @ Wq_rope     # [D, H*RD]
    Wfk = Wkv_down @ Wk_up       # [D, H*ND]
    Wfv = Wkv_down @ Wv_up       # [D, H*HD]

    # per-batch pre-tiled x^T: [p, ic, dt, s]
    xts = [np.ascontiguousarray(
        x[b].reshape(NCHUNK, CHUNK, DT, P).transpose(3, 0, 2, 1)
    ).reshape(P, -1).astype(_BF16) for b in range(B)]

    in_maps = []
    for c in range(8):
        b, g = divmod(c, 4)
        heads = range(HLOC * g, HLOC * (g + 1))
        Wqk = np.empty((D, NCT * P), np.float32)
        for i, h in enumerate(heads):
            q0, k0 = i * HD, HLOC * HD + i * HD
            Wqk[:, q0:q0 + ND] = Wfq[:, h * ND:(h + 1) * ND]
            Wqk[:, q0 + ND:q0 + HD] = Wfqr[:, h * RD:(h + 1) * RD]
            Wqk[:, k0:k0 + ND] = Wfk[:, h * ND:(h + 1) * ND]
            Wqk[:, k0 + ND:k0 + HD] = Wk_rope[:, h * RD:(h + 1) * RD]
        Wv = Wfv[:, g * HLOC * HD:(g + 1) * HLOC * HD]
        Wop = Wo[g * HLOC * HD:(g + 1) * HLOC * HD, :]
        in_maps.append({
            "xt": xts[b],
            "wqk": np.ascontiguousarray(
                Wqk.reshape(DT, P, NCT, P).transpose(1, 2, 0, 3)
            ).reshape(P, -1).astype(_BF16),
            "wv": np.ascontiguousarray(
                Wv.reshape(DT, P, HLOC * HD).transpose(1, 0, 2)
            ).reshape(P, -1).astype(_BF16),
            "wo": np.ascontiguousarray(
                Wop.reshape(HLOC, P, D).transpose(1, 0, 2)
            ).reshape(P, -1).astype(_BF16),
            "cosr": cosr,
            "sinr": sinr,
            "maskd": maskd,
        })
    return in_maps


def kernel(x, Wq_down, Wq_up, Wq_rope, Wkv_down, Wk_up, Wk_rope, Wv_up, Wo,
           _trace=False, _trace_kwargs=None):
    x = np.asarray(x, dtype=np.float32)
    args = [np.asarray(a, dtype=np.float32) for a in
            (Wq_down, Wq_up, Wq_rope, Wkv_down, Wk_up, Wk_rope, Wv_up, Wo)]
    in_maps = _host_prep(x, *args)
    nc = _get_nc()
    res = run_bass_kernel_spmd(nc, in_maps, core_ids=list(range(8)),
                               trace=_trace, **(_trace_kwargs or {}))
    kernel._last_results = res
    out = np.zeros((B, S, D), np.float32)
    for c in range(8):
        # un-tile [p, st, d] -> [st*128+p, d]
        part = res.results[c]["o_part"].reshape(P, S // P, D)
        out[c // 4] += part.transpose(1, 0, 2).reshape(S, D).astype(np.float32)
    return out


# revision 26
# speedup vs baseline: 1.2506x; 1.0138x over previous
"""MLA attention (DeepSeek-style) Trainium2 Bass kernel, 8-core SPMD.

Sharding: core c handles batch b = c//4 and head-group g = c%4 (4 of 16 heads).
All low-rank projections are fused on the host (Wq_down@Wq_up etc.), so every
core runs a single head-parallel projection x @ Wqk [D, 1024] (per-head
[q_nope|q_pe] / [k_nope|k_pe] column tiles) + x @ Wv [D, 512] with ZERO
replicated work, then causal flash attention for its 4 heads and a partial
o-projection. Host sums the 4 partial o-projections per batch.

Device dataflow (per core, transposed-activation layout, S processed in 4
chunks of 512):
  xT (host-tiled, bf16) -> per-head qT/kT [128=HD, S] bf16 tiles straight from
  PSUM (nope rows 0:64, rope rows 64:128), RoPE via host-baked cos/sin tables
  -> V in natural layout via swapped-operand matmuls -> causal flash attention
  per head: scoresT [j,i] matmuls with diagonal tiles shrunk to the unmasked
  query range, exp on ScalarE (scale fused) issued one (head,jt) pair ahead of
  its ones/av consumers (software pipelining - no head-boundary PE bubbles),
  unnormalized attnout + ones-matmul row sums, normalize by broadcast
  reciprocal -> o-projection -> partial [S, D] bf16 out (pre-tiled layout).

Every DMA batch gets its own SBUF tile (dependency tracking is per-tile, so
shared tiles would serialize consumers on the LAST dma). ALL loads are issued
up front: mid-compute DMA measurably slows matmuls via SBUF port contention,
so paying ~20us of saturated-DMA startup buys a clean compute window.
"""

import numpy as np
import ml_dtypes

import concourse.bacc as bacc
import concourse.mybir as mybir
import concourse.tile as tile
from concourse.bass_utils import run_bass_kernel_spmd

F32 = mybir.dt.float32
BF16 = mybir.dt.bfloat16

B, S, D = 2, 2048, 2048
H, HD = 16, 128
RD, ND = 64, 64
KVR, QR = 512, 1024
BASE = 10000.0
HLOC = 4                 # heads per core
CHUNK = 512
NCHUNK = S // CHUNK      # 4
P = 128
DT = D // P              # 16 contraction tiles
NCT = 2 * HLOC           # 8 projection c-tiles (4 q heads + 4 k heads)
SCALE = HD ** -0.5

_BF16 = ml_dtypes.bfloat16


def _build():
    nc = bacc.Bacc("TRN2", target_bir_lowering=False, debug=False)

    xt = nc.dram_tensor("xt", [P, NCHUNK * DT * CHUNK], BF16,
                        kind="ExternalInput").ap()
    wqk = nc.dram_tensor("wqk", [P, NCT * DT * P], BF16,
                         kind="ExternalInput").ap()
    wv = nc.dram_tensor("wv", [P, DT * HLOC * HD], BF16,
                        kind="ExternalInput").ap()
    wo = nc.dram_tensor("wo", [P, HLOC * D], BF16, kind="ExternalInput").ap()
    cosr = nc.dram_tensor("cosr", [RD, S], F32, kind="ExternalInput").ap()
    sinr = nc.dram_tensor("sinr", [RD, S], F32, kind="ExternalInput").ap()
    maskd = nc.dram_tensor("maskd", [P, P], BF16, kind="ExternalInput").ap()
    # output pre-tiled [p, st, d]: fat 4KB-per-partition DMA descriptors
    o_part = nc.dram_tensor("o_part", [P, (S // P) * D], BF16,
                            kind="ExternalOutput").ap()

    xt_r = xt.rearrange("p (ic hf dt s) -> p ic hf dt s",
                        ic=NCHUNK, hf=2, dt=DT // 2)
    xt_r2 = xt.rearrange("p (ic qf dt s) -> p ic qf dt s",
                         ic=NCHUNK, qf=4, dt=DT // 4)
    wqk_r = wqk.rearrange("p (ct dt c) -> p ct dt c", ct=NCT, dt=DT)
    wv_r = wv.rearrange("p (hf dt c) -> p hf dt c", hf=2, dt=DT // 2)
    wo_r = wo.rearrange("p (kt d) -> p kt d", kt=HLOC)
    o_r = o_part.rearrange("p (st d) -> p st d", st=S // P)  # [128, 16, 2048]

    with tile.TileContext(nc) as tc:
        with (
            tc.tile_pool(name="persist", bufs=1) as pp,
            tc.tile_pool(name="acts", bufs=2) as ap_,
            tc.tile_pool(name="rope", bufs=2) as rp,
            tc.tile_pool(name="attn", bufs=3) as atp,
            tc.tile_pool(name="outp", bufs=2) as op_,
            tc.tile_pool(name="aoutp", bufs=2) as aop,
            tc.tile_pool(name="psA", bufs=3, space="PSUM") as psA,
            tc.tile_pool(name="psS", bufs=3, space="PSUM") as psS,
            tc.tile_pool(name="psO", bufs=2, space="PSUM") as psO,
        ):
            # ---------------- persistent tiles (one per DMA batch) ----------
            kTs = [pp.tile([P, S], BF16, tag=f"kT{h}", name=f"kT{h}")
                   for h in range(HLOC)]
            vnat = pp.tile([P, S // P, HLOC * HD], BF16, tag="vnat")
            masks = pp.tile([P, P], BF16, tag="masks")
            ones = pp.tile([P, P], BF16, tag="ones")
            wqkts = [pp.tile([P, DT, P], BF16, tag=f"wqk{ct}", name=f"wqk{ct}")
                     for ct in range(NCT)]
            wvts = [pp.tile([P, DT // 2, HLOC * HD], BF16, tag=f"wv{i}",
                            name=f"wv{i}") for i in range(2)]
            wots = [pp.tile([P, D], BF16, tag=f"wo{kt}", name=f"wo{kt}")
                    for kt in range(HLOC)]
            cos_t = pp.tile([P, S], F32, tag="cos")
            sin_t = pp.tile([P, S], F32, tag="sin")
            dummy = pp.tile([P, 4], BF16, tag="dummy")

            nc.vector.memset(ones[:], 1.0)

            def o_proj(ic, aouts):
                for st in range(CHUNK // P):
                    osb = op_.tile([P, D // CHUNK, CHUNK], BF16, tag="osb")
                    for dc in range(D // CHUNK):
                        ps = psA.tile([P, CHUNK], F32, tag="psA")
                        for kt in range(HLOC):
                            nc.tensor.matmul(
                                ps[:], aouts[kt][:, P * st:P * (st + 1)],
                                wots[kt][:, CHUNK * dc:CHUNK * (dc + 1)],
                                start=(kt == 0), stop=(kt == HLOC - 1))
                        if dc % 2 == 0:
                            nc.scalar.copy(osb[:, dc, :], ps[:])
                        else:
                            nc.vector.tensor_copy(osb[:, dc, :], ps[:])
                    nc.scalar.dma_start(
                        o_r[:, ic * (CHUNK // P) + st, :], osb[:])

            def rope_store(ps_pe, dst_pe, cos_c, sin_c):
                """ps_pe: [64, CHUNK] psum AP at partition base 64 (pre-rope pe
                rows of one head). 4 DVE ops; sign-baked sin tables make the
                NeoX rotation a mult/mult/mult/add. dst_pe = rows [64:128]."""
                b = 64
                tmp = rp.tile([P, CHUNK], F32, tag="ropetmp")
                scr = rp.tile([P, CHUNK], F32, tag="ropescr")
                nc.vector.tensor_tensor(tmp[b:b + 32, :], ps_pe[32:64, :],
                                        sin_c[b:b + 32, :], mybir.AluOpType.mult)
                nc.vector.tensor_tensor(tmp[b + 32:b + 64, :], ps_pe[0:32, :],
                                        sin_c[b + 32:b + 64, :],
                                        mybir.AluOpType.mult)
                nc.vector.tensor_tensor(scr[b:b + 64, :], ps_pe[:],
                                        cos_c[b:b + 64, :], mybir.AluOpType.mult)
                nc.vector.tensor_tensor(dst_pe, scr[b:b + 64, :],
                                        tmp[b:b + 64, :], mybir.AluOpType.add)

            # ---------------- chunk loop ----------------
            for ic in range(NCHUNK):
                sl = slice(ic * CHUNK, (ic + 1) * CHUNK)

                if ic == 0:
                    xcs = [ap_.tile([P, DT // 4, CHUNK], BF16, tag=f"xc{i}",
                                    name=f"xc{i}") for i in range(4)]
                    for i in range(4):
                        nc.sync.dma_start(xcs[i][:], xt_r2[:, ic, i])
                    next_xcs = [ap_.tile([P, DT // 4, CHUNK], BF16,
                                         tag=f"xc{i}", name=f"xc{i}")
                                for i in range(4)]
                else:
                    xcs = cur_xcs
                if 1 <= ic < NCHUNK - 1:
                    next_xcs = [ap_.tile([P, DT // 4, CHUNK], BF16,
                                         tag=f"xc{i}", name=f"xc{i}")
                                for i in range(4)]
                    for i in range(4):
                        nc.sync.dma_start(next_xcs[i][:], xt_r2[:, ic + 1, i])
                cur_xcs = next_xcs

                def xsl(dt_, cols=slice(None), xcs=xcs):
                    return xcs[dt_ // (DT // 4)][:, dt_ % (DT // 4), cols]

                if ic == 0:
                    # critical set races exclusively: xc0/xc1 (sync ring),
                    # wqk strips 0-1 (gpsimd ring), rope tables (scalar ring)
                    nc.gpsimd.dma_start(wqkts[0][:], wqk_r[:, 0])
                    nc.scalar.dma_start(cos_t[64:128, :], cosr[:])
                    nc.scalar.dma_start(sin_t[64:128, :], sinr[:])
                    nc.scalar.dma_start(masks[:], maskd[:])
                    # bulk (9.5MB) gated behind xcq0 arrival via a dummy copy:
                    # it cannot steal SDMA packet slots from the critical set
                    nc.scalar.copy(dummy[0:1, 0:1], xcs[0][0:1, 0, 0:1])
                    for c2 in range(1, NCT):
                        nc.scalar.dma_start(wqkts[c2][:], wqk_r[:, c2])
                    nc.scalar.dma_start(wvts[0][:], wv_r[:, 0])
                    nc.scalar.dma_start(wvts[1][:], wv_r[:, 1])

                cos_c = cos_t[:, sl]
                sin_c = sin_t[:, sl]

                # ---- projection: c-tile ct = head [nope64 | pe64] ----
                # ct 0..3 -> q heads, ct 4..7 -> k heads (identical rope)
                qTis = [ap_.tile([P, CHUNK], BF16, tag=f"qTi{h}",
                                 name=f"qTi{h}") for h in range(HLOC)]
                for ct in range(NCT):
                    ps = psA.tile([P, CHUNK], F32, tag="psA")
                    for dt_ in range(DT):
                        nc.tensor.matmul(
                            ps[:], wqkts[ct][:, dt_, :], xsl(dt_),
                            start=(dt_ == 0), stop=(dt_ == DT - 1))
                    if ct < HLOC:
                        dst_nope = qTis[ct][0:64, :]
                        dst_pe = qTis[ct][64:128, :]
                    else:
                        dst_nope = kTs[ct - HLOC][0:64, sl]
                        dst_pe = kTs[ct - HLOC][64:128, sl]
                    nc.vector.tensor_copy(dst_nope, ps[0:64, :])
                    rope_store(ps[64:128, :], dst_pe, cos_c, sin_c)

                if ic == 0:
                    # Wo + chunk-1 x, tail of the gated bulk (scalar FIFO)
                    for i in range(4):
                        nc.scalar.dma_start(next_xcs[i][:], xt_r2[:, 1, i])
                    for kt in range(HLOC):
                        nc.scalar.dma_start(wots[kt][:], wo_r[:, kt])

                # ---- v natural [CHUNK, 512]: x seq-tile stationary ----
                for st in range(CHUNK // P):
                    ps = psA.tile([P, HLOC * HD], F32, tag="psA")
                    for dt_ in range(DT):
                        nc.tensor.matmul(
                            ps[:], xsl(dt_, slice(P * st, P * (st + 1))),
                            wvts[dt_ // (DT // 2)][:, dt_ % (DT // 2), :],
                            start=(dt_ == 0), stop=(dt_ == DT - 1))
                    nc.vector.tensor_copy(vnat[:, ic * (CHUNK // P) + st, :],
                                          ps[:])

                # ---- o-projection of the PREVIOUS chunk: PE work to cover
                # the DVE rope/normalize backlog of this chunk's projections
                if ic > 0:
                    o_proj(ic - 1, prev_aouts)

                # ---- attention for this query chunk ----
                # diagonal j-tiles shrink to queries >= P*r (the rest are
                # fully masked and contribute exact zeros by omission);
                # score+exp issue one (h, jt) pair ahead of ones/av
                aouts = [aop.tile([P, CHUNK], BF16, tag=f"aout{h}",
                                  name=f"aout{h}") for h in range(HLOC)]
                jt_max = (ic + 1) * (CHUNK // P)

                def issue_score(h, jt):
                    r = jt - ic * (CHUNK // P)
                    q0 = P * r if r > 0 else 0
                    pss = psS.tile([P, CHUNK], F32, tag="psS")
                    nc.tensor.matmul(
                        pss[:, q0:], kTs[h][:, P * jt:P * (jt + 1)],
                        qTis[h][:, q0:], start=True, stop=True)
                    at = atp.tile([P, CHUNK], BF16, tag="attnT")
                    nc.scalar.activation(
                        at[:, q0:], pss[:, q0:],
                        mybir.ActivationFunctionType.Exp, scale=SCALE)
                    if r >= 0:  # triangular mask on the diagonal subtile
                        nc.vector.tensor_tensor(
                            at[:, q0:q0 + P], at[:, q0:q0 + P], masks[:],
                            mybir.AluOpType.mult)
                    return at, q0

                pairs = [(h, jt) for h in range(HLOC) for jt in range(jt_max)]
                pending = {pairs[0]: issue_score(*pairs[0])}
                psd = pso = None
                for idx, (h, jt) in enumerate(pairs):
                    if idx + 1 < len(pairs):
                        pending[pairs[idx + 1]] = issue_score(*pairs[idx + 1])
                    at, q0 = pending.pop((h, jt))
                    if jt == 0:
                        psd = psA.tile([P, CHUNK], F32, tag="psA")
                        pso = psO.tile([P, CHUNK], F32, tag="psO")
                    nc.tensor.matmul(psd[:, q0:], ones[:], at[:, q0:],
                                     start=(jt == 0), stop=(jt == jt_max - 1))
                    nc.tensor.matmul(
                        pso[:, q0:], vnat[:, jt, HD * h:HD * (h + 1)],
                        at[:, q0:],
                        start=(jt == 0), stop=(jt == jt_max - 1))
                    if jt == jt_max - 1:
                        rec = atp.tile([P, CHUNK], F32, tag="recip")
                        nc.vector.reciprocal_approx_fast(rec[:], psd[:])
                        nc.vector.tensor_tensor(aouts[h][:], pso[:], rec[:],
                                                mybir.AluOpType.mult)
                prev_aouts = aouts

            o_proj(NCHUNK - 1, prev_aouts)
    nc.compile()
    return nc


_NC = None


def _get_nc():
    global _NC
    if _NC is None:
        _NC = _build()
    return _NC


def _host_prep(x, Wq_down, Wq_up, Wq_rope, Wkv_down, Wk_up, Wk_rope, Wv_up, Wo):
    """Build the 8 per-core input maps (all host-side layout prep)."""
    # rope tables for SBUF partition rows 64:128 (the pe rows), NeoX sign
    # baked into sin
    half = RD // 2
    inv_freq = 1.0 / (BASE ** (np.arange(half, dtype=np.float64) / half))
    ang = np.arange(S, dtype=np.float64)[None, :] * inv_freq[:, None]  # [32, S]
    cos32 = np.cos(ang)
    sin32 = np.sin(ang)
    cosr = np.tile(cos32, (2, 1)).astype(np.float32)                   # [64,S]
    sinr = np.concatenate([-sin32, sin32], 0).astype(np.float32)

    # triangular mask for the 128x128 diagonal subtile: key p <= query i
    pidx = np.arange(P)[:, None]
    iidx = np.arange(P)[None, :]
    maskd = (pidx <= iidx).astype(_BF16)

    # fuse the low-rank compositions once, in f32
    Wfq = Wq_down @ Wq_up        # [D, H*ND]
    Wfqr = Wq_down @ Wq_rope     # [D, H*RD]
    Wfk = Wkv_down @ Wk_up       # [D, H*ND]
    Wfv = Wkv_down @ Wv_up       # [D, H*HD]

    # per-batch pre-tiled x^T: [p, ic, dt, s]
    xts = [np.ascontiguousarray(
        x[b].reshape(NCHUNK, CHUNK, DT, P).transpose(3, 0, 2, 1)
    ).reshape(P, -1).astype(_BF16) for b in range(B)]

    in_maps = []
    for c in range(8):
        b, g = divmod(c, 4)
        heads = range(HLOC * g, HLOC * (g + 1))
        Wqk = np.empty((D, NCT * P), np.float32)
        for i, h in enumerate(heads):
            q0, k0 = i * HD, HLOC * HD + i * HD
            Wqk[:, q0:q0 + ND] = Wfq[:, h * ND:(h + 1) * ND]
            Wqk[:, q0 + ND:q0 + HD] = Wfqr[:, h * RD:(h + 1) * RD]
            Wqk[:, k0:k0 + ND] = Wfk[:, h * ND:(h + 1) * ND]
            Wqk[:, k0 + ND:k0 + HD] = Wk_rope[:, h * RD:(h + 1) * RD]
        Wv = Wfv[:, g * HLOC * HD:(g + 1) * HLOC * HD]
        Wop = Wo[g * HLOC * HD:(g + 1) * HLOC * HD, :]
        in_maps.append({
            "xt": xts[b],
            "wqk": np.ascontiguousarray(
                Wqk.reshape(DT, P, NCT, P).transpose(1, 2, 0, 3)
            ).reshape(P, -1).astype(_BF16),
            "wv": np.ascontiguousarray(
                Wv.reshape(DT, P, HLOC * HD).transpose(1, 0, 2)
            ).reshape(P, -1).astype(_BF16),
            "wo": np.ascontiguousarray(
                Wop.reshape(HLOC, P, D).transpose(1, 0, 2)
            ).reshape(P, -1).astype(_BF16),
            "cosr": cosr,
            "sinr": sinr,
            "maskd": maskd,
        })
    return in_maps


def kernel(x, Wq_down, Wq_up, Wq_rope, Wkv_down, Wk_up, Wk_rope, Wv_up, Wo,
           _trace=False, _trace_kwargs=None):
    x = np.asarray(x, dtype=np.float32)
    args = [np.asarray(a, dtype=np.float32) for a in
            (Wq_down, Wq_up, Wq_rope, Wkv_down, Wk_up, Wk_rope, Wv_up, Wo)]
    in_maps = _host_prep(x, *args)
    nc = _get_nc()
    res = run_bass_kernel_spmd(nc, in_maps, core_ids=list(range(8)),
                               trace=_trace, **(_trace_kwargs or {}))
    kernel._last_results = res
    out = np.zeros((B, S, D), np.float32)
    for c in range(8):
        # un-tile [p, st, d] -> [st*128+p, d]
        part = res.results[c]["o_part"].reshape(P, S // P, D)
        out[c // 4] += part.transpose(1, 0, 2).reshape(S, D).astype(np.float32)
    return out


# revision 27
# speedup vs baseline: 1.2529x; 1.0018x over previous
"""MLA attention (DeepSeek-style) Trainium2 Bass kernel, 8-core SPMD.

Sharding: core c handles batch b = c//4 and head-group g = c%4 (4 of 16 heads).
All low-rank projections are fused on the host (Wq_down@Wq_up etc.), so every
core runs a single head-parallel projection x @ Wqk [D, 1024] (per-head
[q_nope|q_pe] / [k_nope|k_pe] column tiles) + x @ Wv [D, 512] with ZERO
replicated work, then causal flash attention for its 4 heads and a partial
o-projection. Host sums the 4 partial o-projections per batch.

Device dataflow (per core, transposed-activation layout, S processed in 4
chunks of 512):
  xT (host-tiled, bf16) -> per-head qT/kT [128=HD, S] bf16 tiles straight from
  PSUM (nope rows 0:64, rope rows 64:128), RoPE via host-baked cos/sin tables
  -> V in natural layout via swapped-operand matmuls -> causal flash attention
  per head: scoresT [j,i] matmuls with diagonal tiles shrunk to the unmasked
  query range, exp on ScalarE (scale fused) issued one (head,jt) pair ahead of
  its ones/av consumers (software pipelining - no head-boundary PE bubbles),
  unnormalized attnout + ones-matmul row sums, normalize by broadcast
  reciprocal -> o-projection -> partial [S, D] bf16 out (pre-tiled layout).

Every DMA batch gets its own SBUF tile (dependency tracking is per-tile, so
shared tiles would serialize consumers on the LAST dma). ALL loads are issued
up front: mid-compute DMA measurably slows matmuls via SBUF port contention,
so paying ~20us of saturated-DMA startup buys a clean compute window.
"""

import numpy as np
import ml_dtypes

import concourse.bacc as bacc
import concourse.mybir as mybir
import concourse.tile as tile
from concourse.bass_utils import run_bass_kernel_spmd

F32 = mybir.dt.float32
BF16 = mybir.dt.bfloat16

B, S, D = 2, 2048, 2048
H, HD = 16, 128
RD, ND = 64, 64
KVR, QR = 512, 1024
BASE = 10000.0
HLOC = 4                 # heads per core
CHUNK = 512
NCHUNK = S // CHUNK      # 4
P = 128
DT = D // P              # 16 contraction tiles
NCT = 2 * HLOC           # 8 projection c-tiles (4 q heads + 4 k heads)
SCALE = HD ** -0.5

_BF16 = ml_dtypes.bfloat16


def _build():
    nc = bacc.Bacc("TRN2", target_bir_lowering=False, debug=False)

    xt = nc.dram_tensor("xt", [P, NCHUNK * DT * CHUNK], BF16,
                        kind="ExternalInput").ap()
    wqk = nc.dram_tensor("wqk", [P, NCT * DT * P], BF16,
                         kind="ExternalInput").ap()
    wv = nc.dram_tensor("wv", [P, DT * HLOC * HD], BF16,
                        kind="ExternalInput").ap()
    wo = nc.dram_tensor("wo", [P, HLOC * D], BF16, kind="ExternalInput").ap()
    cosr = nc.dram_tensor("cosr", [RD, S], F32, kind="ExternalInput").ap()
    sinr = nc.dram_tensor("sinr", [RD, S], F32, kind="ExternalInput").ap()
    maskd = nc.dram_tensor("maskd", [P, P], BF16, kind="ExternalInput").ap()
    # output pre-tiled [p, st, d]: fat 4KB-per-partition DMA descriptors
    o_part = nc.dram_tensor("o_part", [P, (S // P) * D], BF16,
                            kind="ExternalOutput").ap()

    xt_r = xt.rearrange("p (ic hf dt s) -> p ic hf dt s",
                        ic=NCHUNK, hf=2, dt=DT // 2)
    xt_r2 = xt.rearrange("p (ic qf dt s) -> p ic qf dt s",
                         ic=NCHUNK, qf=4, dt=DT // 4)
    wqk_r = wqk.rearrange("p (ct dt c) -> p ct dt c", ct=NCT, dt=DT)
    wv_r = wv.rearrange("p (hf dt c) -> p hf dt c", hf=2, dt=DT // 2)
    wo_r = wo.rearrange("p (kt d) -> p kt d", kt=HLOC)
    o_r = o_part.rearrange("p (st d) -> p st d", st=S // P)  # [128, 16, 2048]

    with tile.TileContext(nc) as tc:
        with (
            tc.tile_pool(name="persist", bufs=1) as pp,
            tc.tile_pool(name="acts", bufs=2) as ap_,
            tc.tile_pool(name="rope", bufs=2) as rp,
            tc.tile_pool(name="attn", bufs=3) as atp,
            tc.tile_pool(name="outp", bufs=2) as op_,
            tc.tile_pool(name="aoutp", bufs=2) as aop,
            tc.tile_pool(name="psA", bufs=3, space="PSUM") as psA,
            tc.tile_pool(name="psS", bufs=3, space="PSUM") as psS,
            tc.tile_pool(name="psO", bufs=2, space="PSUM") as psO,
        ):
            # ---------------- persistent tiles (one per DMA batch) ----------
            kTs = [pp.tile([P, S], BF16, tag=f"kT{h}", name=f"kT{h}")
                   for h in range(HLOC)]
            vnat = pp.tile([P, S // P, HLOC * HD], BF16, tag="vnat")
            masks = pp.tile([P, P], BF16, tag="masks")
            ones = pp.tile([P, P], BF16, tag="ones")
            wqkts = [pp.tile([P, DT, P], BF16, tag=f"wqk{ct}", name=f"wqk{ct}")
                     for ct in range(NCT)]
            wvts = [pp.tile([P, DT // 2, HLOC * HD], BF16, tag=f"wv{i}",
                            name=f"wv{i}") for i in range(2)]
            wots = [pp.tile([P, D], BF16, tag=f"wo{kt}", name=f"wo{kt}")
                    for kt in range(HLOC)]
            cos_t = pp.tile([P, S], F32, tag="cos")
            sin_t = pp.tile([P, S], F32, tag="sin")
            dummy = pp.tile([P, 4], BF16, tag="dummy")

            nc.vector.memset(ones[:], 1.0)

            def o_proj(ic, aouts):
                for st in range(CHUNK // P):
                    osb = op_.tile([P, D // CHUNK, CHUNK], BF16, tag="osb")
                    for dc in range(D // CHUNK):
                        ps = psA.tile([P, CHUNK], F32, tag="psA")
                        for kt in range(HLOC):
                            nc.tensor.matmul(
                                ps[:], aouts[kt][:, P * st:P * (st + 1)],
                                wots[kt][:, CHUNK * dc:CHUNK * (dc + 1)],
                                start=(kt == 0), stop=(kt == HLOC - 1))
                        if dc % 2 == 0:
                            nc.scalar.copy(osb[:, dc, :], ps[:])
                        else:
                            nc.vector.tensor_copy(osb[:, dc, :], ps[:])
                    nc.scalar.dma_start(
                        o_r[:, ic * (CHUNK // P) + st, :], osb[:])

            def rope_store(ps_pe, dst_pe, cos_c, sin_c):
                """ps_pe: [64, CHUNK] psum AP at partition base 64 (pre-rope pe
                rows of one head). 4 DVE ops; sign-baked sin tables make the
                NeoX rotation a mult/mult/mult/add. dst_pe = rows [64:128]."""
                b = 64
                tmp = rp.tile([P, CHUNK], F32, tag="ropetmp")
                scr = rp.tile([P, CHUNK], F32, tag="ropescr")
                nc.vector.tensor_tensor(tmp[b:b + 32, :], ps_pe[32:64, :],
                                        sin_c[b:b + 32, :], mybir.AluOpType.mult)
                nc.vector.tensor_tensor(tmp[b + 32:b + 64, :], ps_pe[0:32, :],
                                        sin_c[b + 32:b + 64, :],
                                        mybir.AluOpType.mult)
                nc.vector.tensor_tensor(scr[b:b + 64, :], ps_pe[:],
                                        cos_c[b:b + 64, :], mybir.AluOpType.mult)
                nc.vector.tensor_tensor(dst_pe, scr[b:b + 64, :],
                                        tmp[b:b + 64, :], mybir.AluOpType.add)

            # ---------------- chunk loop ----------------
            for ic in range(NCHUNK):
                sl = slice(ic * CHUNK, (ic + 1) * CHUNK)

                if ic == 0:
                    xcs = [ap_.tile([P, DT // 4, CHUNK], BF16, tag=f"xc{i}",
                                    name=f"xc{i}") for i in range(4)]
                    for i in range(4):
                        nc.sync.dma_start(xcs[i][:], xt_r2[:, ic, i])
                    next_xcs = [ap_.tile([P, DT // 4, CHUNK], BF16,
                                         tag=f"xc{i}", name=f"xc{i}")
                                for i in range(4)]
                else:
                    xcs = cur_xcs
                if 1 <= ic < NCHUNK - 1:
                    next_xcs = [ap_.tile([P, DT // 4, CHUNK], BF16,
                                         tag=f"xc{i}", name=f"xc{i}")
                                for i in range(4)]
                    for i in range(4):
                        nc.sync.dma_start(next_xcs[i][:], xt_r2[:, ic + 1, i])
                cur_xcs = next_xcs

                def xsl(dt_, cols=slice(None), xcs=xcs):
                    return xcs[dt_ // (DT // 4)][:, dt_ % (DT // 4), cols]

                if ic == 0:
                    # critical set races exclusively: xc0/xc1 (sync ring),
                    # wqk strips 0-1 (gpsimd ring), rope tables (scalar ring)
                    nc.gpsimd.dma_start(wqkts[0][:], wqk_r[:, 0])
                    nc.scalar.dma_start(cos_t[64:128, :], cosr[:])
                    nc.scalar.dma_start(sin_t[64:128, :], sinr[:])
                    nc.scalar.dma_start(masks[:], maskd[:])
                    # bulk (9.5MB) gated behind xcq0 arrival via a dummy copy:
                    # it cannot steal SDMA packet slots from the critical set
                    nc.scalar.copy(dummy[0:1, 0:1], xcs[0][0:1, 0, 0:1])
                    for c2 in range(1, NCT):
                        nc.scalar.dma_start(wqkts[c2][:], wqk_r[:, c2])
                    nc.scalar.dma_start(wvts[0][:], wv_r[:, 0])
                    nc.scalar.dma_start(wvts[1][:], wv_r[:, 1])

                cos_c = cos_t[:, sl]
                sin_c = sin_t[:, sl]

                # ---- projection: c-tile ct = head [nope64 | pe64] ----
                # ct 0..3 -> q heads, ct 4..7 -> k heads (identical rope)
                qTis = [ap_.tile([P, CHUNK], BF16, tag=f"qTi{h}",
                                 name=f"qTi{h}") for h in range(HLOC)]
                for ct in (range(NCT) if ic == 0 else range(HLOC)):
                    ps = psA.tile([P, CHUNK], F32, tag="psA")
                    for dt_ in range(DT):
                        nc.tensor.matmul(
                            ps[:], wqkts[ct][:, dt_, :], xsl(dt_),
                            start=(dt_ == 0), stop=(dt_ == DT - 1))
                    if ct < HLOC:
                        dst_nope = qTis[ct][0:64, :]
                        dst_pe = qTis[ct][64:128, :]
                    else:
                        dst_nope = kTs[ct - HLOC][0:64, sl]
                        dst_pe = kTs[ct - HLOC][64:128, sl]
                    nc.vector.tensor_copy(dst_nope, ps[0:64, :])
                    rope_store(ps[64:128, :], dst_pe, cos_c, sin_c)

                if ic == 0:
                    # Wo + chunk-1 x, tail of the gated bulk (scalar FIFO)
                    for i in range(4):
                        nc.scalar.dma_start(next_xcs[i][:], xt_r2[:, 1, i])
                    for kt in range(HLOC):
                        nc.scalar.dma_start(wots[kt][:], wo_r[:, kt])

                # ---- v natural [CHUNK, 512]: x seq-tile stationary ----
                for st in range(CHUNK // P):
                    ps = psA.tile([P, HLOC * HD], F32, tag="psA")
                    for dt_ in range(DT):
                        nc.tensor.matmul(
                            ps[:], xsl(dt_, slice(P * st, P * (st + 1))),
                            wvts[dt_ // (DT // 2)][:, dt_ % (DT // 2), :],
                            start=(dt_ == 0), stop=(dt_ == DT - 1))
                    nc.vector.tensor_copy(vnat[:, ic * (CHUNK // P) + st, :],
                                          ps[:])

                # ---- o-projection of the PREVIOUS chunk: PE work to cover
                # the DVE rope/normalize backlog of this chunk's projections
                if ic > 0:
                    o_proj(ic - 1, prev_aouts)

                # ---- attention for this query chunk ----
                # diagonal j-tiles shrink to queries >= P*r (the rest are
                # fully masked and contribute exact zeros by omission);
                # score+exp issue one (h, jt) pair ahead of ones/av
                aouts = [aop.tile([P, CHUNK], BF16, tag=f"aout{h}",
                                  name=f"aout{h}") for h in range(HLOC)]
                jt_max = (ic + 1) * (CHUNK // P)

                def issue_score(h, jt):
                    r = jt - ic * (CHUNK // P)
                    q0 = P * r if r > 0 else 0
                    pss = psS.tile([P, CHUNK], F32, tag="psS")
                    nc.tensor.matmul(
                        pss[:, q0:], kTs[h][:, P * jt:P * (jt + 1)],
                        qTis[h][:, q0:], start=True, stop=True)
                    at = atp.tile([P, CHUNK], BF16, tag="attnT")
                    nc.scalar.activation(
                        at[:, q0:], pss[:, q0:],
                        mybir.ActivationFunctionType.Exp, scale=SCALE)
                    if r >= 0:  # triangular mask on the diagonal subtile
                        nc.vector.tensor_tensor(
                            at[:, q0:q0 + P], at[:, q0:q0 + P], masks[:],
                            mybir.AluOpType.mult)
                    return at, q0

                pairs = [(h, jt) for h in range(HLOC) for jt in range(jt_max)]
                pending = {pairs[0]: issue_score(*pairs[0])}
                psd = pso = None
                for idx, (h, jt) in enumerate(pairs):
                    if idx + 1 < len(pairs):
                        pending[pairs[idx + 1]] = issue_score(*pairs[idx + 1])
                    at, q0 = pending.pop((h, jt))
                    if jt == 0:
                        psd = psA.tile([P, CHUNK], F32, tag="psA")
                        pso = psO.tile([P, CHUNK], F32, tag="psO")
                    nc.tensor.matmul(psd[:, q0:], ones[:], at[:, q0:],
                                     start=(jt == 0), stop=(jt == jt_max - 1))
                    nc.tensor.matmul(
                        pso[:, q0:], vnat[:, jt, HD * h:HD * (h + 1)],
                        at[:, q0:],
                        start=(jt == 0), stop=(jt == jt_max - 1))
                    if jt == jt_max - 1:
                        rec = atp.tile([P, CHUNK], F32, tag="recip")
                        nc.vector.reciprocal_approx_fast(rec[:], psd[:])
                        nc.vector.tensor_tensor(aouts[h][:], pso[:], rec[:],
                                                mybir.AluOpType.mult)
                        if ic + 1 < NCHUNK:
                            # next chunk's k projection for this head: PE
                            # filler during the Scalar(exp)-paced attention
                            sl_n = slice((ic + 1) * CHUNK, (ic + 2) * CHUNK)
                            ct = HLOC + h
                            ps = psA.tile([P, CHUNK], F32, tag="psA")
                            for dt_ in range(DT):
                                nc.tensor.matmul(
                                    ps[:], wqkts[ct][:, dt_, :],
                                    cur_xcs[dt_ // (DT // 4)][
                                        :, dt_ % (DT // 4), :],
                                    start=(dt_ == 0), stop=(dt_ == DT - 1))
                            nc.vector.tensor_copy(kTs[h][0:64, sl_n],
                                                  ps[0:64, :])
                            rope_store(ps[64:128, :], kTs[h][64:128, sl_n],
                                       cos_t[:, sl_n], sin_t[:, sl_n])
                prev_aouts = aouts

            o_proj(NCHUNK - 1, prev_aouts)
    nc.compile()
    return nc


_NC = None


def _get_nc():
    global _NC
    if _NC is None:
        _NC = _build()
    return _NC


def _host_prep(x, Wq_down, Wq_up, Wq_rope, Wkv_down, Wk_up, Wk_rope, Wv_up, Wo):
    """Build the 8 per-core input maps (all host-side layout prep)."""
    # rope tables for SBUF partition rows 64:128 (the pe rows), NeoX sign
    # baked into sin
    half = RD // 2
    inv_freq = 1.0 / (BASE ** (np.arange(half, dtype=np.float64) / half))
    ang = np.arange(S, dtype=np.float64)[None, :] * inv_freq[:, None]  # [32, S]
    cos32 = np.cos(ang)
    sin32 = np.sin(ang)
    cosr = np.tile(cos32, (2, 1)).astype(np.float32)                   # [64,S]
    sinr = np.concatenate([-sin32, sin32], 0).astype(np.float32)

    # triangular mask for the 128x128 diagonal subtile: key p <= query i
    pidx = np.arange(P)[:, None]
    iidx = np.arange(P)[None, :]
    maskd = (pidx <= iidx).astype(_BF16)

    # fuse the low-rank compositions once, in f32
    Wfq = Wq_down @ Wq_up        # [D, H*ND]
    Wfqr = Wq_down @ Wq_rope     # [D, H*RD]
    Wfk = Wkv_down @ Wk_up       # [D, H*ND]
    Wfv = Wkv_down @ Wv_up       # [D, H*HD]

    # per-batch pre-tiled x^T: [p, ic, dt, s]
    xts = [np.ascontiguousarray(
        x[b].reshape(NCHUNK, CHUNK, DT, P).transpose(3, 0, 2, 1)
    ).reshape(P, -1).astype(_BF16) for b in range(B)]

    in_maps = []
    for c in range(8):
        b, g = divmod(c, 4)
        heads = range(HLOC * g, HLOC * (g + 1))
        Wqk = np.empty((D, NCT * P), np.float32)
        for i, h in enumerate(heads):
            q0, k0 = i * HD, HLOC * HD + i * HD
            Wqk[:, q0:q0 + ND] = Wfq[:, h * ND:(h + 1) * ND]
            Wqk[:, q0 + ND:q0 + HD] = Wfqr[:, h * RD:(h + 1) * RD]
            Wqk[:, k0:k0 + ND] = Wfk[:, h * ND:(h + 1) * ND]
            Wqk[:, k0 + ND:k0 + HD] = Wk_rope[:, h * RD:(h + 1) * RD]
        Wv = Wfv[:, g * HLOC * HD:(g + 1) * HLOC * HD]
        Wop = Wo[g * HLOC * HD:(g + 1) * HLOC * HD, :]
        in_maps.append({
            "xt": xts[b],
            "wqk": np.ascontiguousarray(
                Wqk.reshape(DT, P, NCT, P).transpose(1, 2, 0, 3)
            ).reshape(P, -1).astype(_BF16),
            "wv": np.ascontiguousarray(
                Wv.reshape(DT, P, HLOC * HD).transpose(1, 0, 2)
            ).reshape(P, -1).astype(_BF16),
            "wo": np.ascontiguousarray(
                Wop.reshape(HLOC, P, D).transpose(1, 0, 2)
            ).reshape(P, -1).astype(_BF16),
            "cosr": cosr,
            "sinr": sinr,
            "maskd": maskd,
        })
    return in_maps


def kernel(x, Wq_down, Wq_up, Wq_rope, Wkv_down, Wk_up, Wk_rope, Wv_up, Wo,
           _trace=False, _trace_kwargs=None):
    x = np.asarray(x, dtype=np.float32)
    args = [np.asarray(a, dtype=np.float32) for a in
            (Wq_down, Wq_up, Wq_rope, Wkv_down, Wk_up, Wk_rope, Wv_up, Wo)]
    in_maps = _host_prep(x, *args)
    nc = _get_nc()
    res = run_bass_kernel_spmd(nc, in_maps, core_ids=list(range(8)),
                               trace=_trace, **(_trace_kwargs or {}))
    kernel._last_results = res
    out = np.zeros((B, S, D), np.float32)
    for c in range(8):
        # un-tile [p, st, d] -> [st*128+p, d]
        part = res.results[c]["o_part"].reshape(P, S // P, D)
        out[c // 4] += part.transpose(1, 0, 2).reshape(S, D).astype(np.float32)
    return out
